# revision 16
# baseline (speedup 1.0000x reference)
"""HAN (hierarchical attention network) forward pass on 8 TRN2 NeuronCores.

Strategy
--------
Data-parallel over batch: each core handles 8 documents = 128 sentences =
4096 tokens, fully independently (no collectives). Inside a core:

* Embedding lookup + word-GRU input projection folded on host:
  gi = (emb @ Wih.T)[tokens], gathered per step with indirect DMA.
* Word bi-GRU, batch-major [128 sentences, feat]. Per-direction gate math
  so the two independent direction recurrences pipeline across ACT/DVE/
  GPSIMD. Engine programs are ordered so the d0 chain (rec matmul ->
  sigmoid -> r*hn -> +gi -> tanh -> blend -> transpose -> copy) never
  waits behind low-priority work; attention and injects fill PE gaps.
* gi injection PSUM groups are single-buffered (inject for t+1 reuses the
  banks right after sigmoid(t) reads them), freeing banks for TWO
  transpose banks (pt0/pt1) so hT copies overlap the second transpose
  pair, and for a RESIDENT weighted-sum accumulator bank (diag(exp(s))
  matmuls accumulate across all 32 steps; no per-4-step PSUM drain).
* Word attention: scores accumulated in-loop (u = tanh(h Wa + ba) lagged
  one step; u.v via stt-accum). Softmax without max-subtraction as an
  exp-weighted running matmul sum.
* Sentence bi-GRU: feature-major (free dims 16-32), the two directions
  emitted staggered: d1's 12 weight-load+matmul pairs stream on PE while
  d0's gate math runs on ACT/DVE, and vice versa. Sentence attention is
  a post-pass batch GEMM; per-document softmax via indicator matmuls.

Compute dtype bf16 (fp32 PSUM accumulation + fp32 attention
accumulators).
"""

import numpy as np
import ml_dtypes

import concourse.bass as bass
import concourse.mybir as mybir
import concourse.tile as tile
from concourse import bacc, bass_utils
from concourse.masks import make_identity

BF = mybir.dt.bfloat16
F32 = mybir.dt.float32
AF = mybir.ActivationFunctionType
ALU = mybir.AluOpType
bf16 = ml_dtypes.bfloat16

V, E = 50000, 300
HW_, HS_ = 256, 256
NCLS = 10
B, S, W = 64, 16, 32
NCORES = 8
BC = B // NCORES          # docs per core = 8
NW = BC * S               # word-level batch per core = 128
GW = 3 * HW_              # 768


def _build_program():
    nc = bacc.Bacc(
        "TRN2",
        target_bir_lowering=False,
        debug=False,
        enable_asserts=False,
        num_devices=NCORES,
    )

    # ---- DRAM I/O ----
    h = {}
    h["G"] = nc.dram_tensor("G", [V, 1536], BF, kind="ExternalInput")
    h["toks"] = nc.dram_tensor("toks", [128, 32], mybir.dt.int32, kind="ExternalInput")
    h["whhT"] = nc.dram_tensor("whhT", [4, 128, GW], BF, kind="ExternalInput")
    h["brow"] = nc.dram_tensor("brow", [1, 512], BF, kind="ExternalInput")
    h["waT"] = nc.dram_tensor("waT", [512, 512], BF, kind="ExternalInput")
    h["barow"] = nc.dram_tensor("barow", [1, 512], BF, kind="ExternalInput")
    h["vb"] = nc.dram_tensor("vb", [128, 512], BF, kind="ExternalInput")
    h["swihT"] = nc.dram_tensor("swihT", [512, 1536], BF, kind="ExternalInput")
    h["sprow"] = nc.dram_tensor("sprow", [1, 1536], BF, kind="ExternalInput")
    h["swhhF"] = nc.dram_tensor("swhhF", [24, 128, 128], BF, kind="ExternalInput")
    h["sbrowF"] = nc.dram_tensor("sbrowF", [4, 128], BF, kind="ExternalInput")
    h["bones"] = nc.dram_tensor("bones", [4, 32], BF, kind="ExternalInput")
    h["sbrow"] = nc.dram_tensor("sbrow", [1, 512], BF, kind="ExternalInput")
    h["sawT"] = nc.dram_tensor("sawT", [512, 512], BF, kind="ExternalInput")
    h["sbarow"] = nc.dram_tensor("sbarow", [1, 512], BF, kind="ExternalInput")
    h["svb"] = nc.dram_tensor("svb", [128, 512], BF, kind="ExternalInput")
    h["ind8"] = nc.dram_tensor("ind8", [128, 8], BF, kind="ExternalInput")
    h["ind8f"] = nc.dram_tensor("ind8f", [128, 8], F32, kind="ExternalInput")
    h["ind8T"] = nc.dram_tensor("ind8T", [8, 128], F32, kind="ExternalInput")
    h["fcwT"] = nc.dram_tensor("fcwT", [512, NCLS], BF, kind="ExternalInput")
    h["fcb"] = nc.dram_tensor("fcb", [1, NCLS], BF, kind="ExternalInput")
    h["out"] = nc.dram_tensor("out", [BC, NCLS], F32, kind="ExternalOutput")

    with tile.TileContext(nc) as tc:
        _body(nc, tc, h)
    nc.compile()
    return nc


def _body(nc, tc, handles):
    def dram(name):
        return handles[name].ap()

    G_ap = dram("G")
    with tc.tile_pool(name="const", bufs=1) as cp:
        # ---- constants / weights in SBUF ----
        ident = cp.tile([128, 128], BF)
        make_identity(nc, ident)
        ones = cp.tile([1, 128], BF)
        nc.gpsimd.memset(ones, 1.0)

        toks = cp.tile([128, 32], mybir.dt.int32)
        nc.sync.dma_start(out=toks, in_=dram("toks"))
        whh = cp.tile([128, 4 * GW], BF)  # (d0k0 d0k1 d1k0 d1k1); [rz(512) n(256)]
        for j in range(4):
            nc.sync.dma_start(out=whh[:, j * GW:(j + 1) * GW],
                              in_=dram("whhT")[j])
        brow = cp.tile([1, 512], BF)
        nc.sync.dma_start(out=brow, in_=dram("brow"))
        waT = cp.tile([128, 4 * 512], BF)
        for j in range(4):
            nc.sync.dma_start(out=waT[:, j * 512:(j + 1) * 512],
                              in_=dram("waT")[j * 128:(j + 1) * 128, :])
        barow = cp.tile([1, 512], BF)
        nc.sync.dma_start(out=barow, in_=dram("barow"))
        vb = cp.tile([128, 512], BF)
        nc.sync.dma_start(out=vb, in_=dram("vb"))

        swihT = cp.tile([128, 4 * 1536], BF)
        for j in range(4):
            nc.sync.dma_start(out=swihT[:, j * 1536:(j + 1) * 1536],
                              in_=dram("swihT")[j * 128:(j + 1) * 128, :])
        sprow = cp.tile([1, 1536], BF)
        nc.sync.dma_start(out=sprow, in_=dram("sprow"))
        swhhF = cp.tile([128, 24 * 128], BF)
        for j in range(24):
            nc.sync.dma_start(out=swhhF[:, j * 128:(j + 1) * 128],
                              in_=dram("swhhF")[j])
        sbrow = cp.tile([1, 512], BF)
        nc.sync.dma_start(out=sbrow, in_=dram("sbrow"))
        sbrowF = cp.tile([4, 128], BF)
        nc.sync.dma_start(out=sbrowF, in_=dram("sbrowF"))
        bones = cp.tile([4, 32], BF)
        nc.sync.dma_start(out=bones, in_=dram("bones"))
        sawT = cp.tile([128, 4 * 512], BF)
        for j in range(4):
            nc.sync.dma_start(out=sawT[:, j * 512:(j + 1) * 512],
                              in_=dram("sawT")[j * 128:(j + 1) * 128, :])
        sbarow = cp.tile([1, 512], BF)
        nc.sync.dma_start(out=sbarow, in_=dram("sbarow"))
        svb = cp.tile([128, 512], BF)
        nc.sync.dma_start(out=svb, in_=dram("svb"))
        ind8 = cp.tile([128, 8], BF)
        nc.sync.dma_start(out=ind8, in_=dram("ind8"))
        ind8f = cp.tile([128, 8], F32)
        nc.sync.dma_start(out=ind8f, in_=dram("ind8f"))
        ind8T = cp.tile([8, 128], F32)
        nc.sync.dma_start(out=ind8T, in_=dram("ind8T"))
        fcwT = cp.tile([128, 4 * NCLS], BF)
        for j in range(4):
            nc.sync.dma_start(out=fcwT[:, j * NCLS:(j + 1) * NCLS],
                              in_=dram("fcwT")[j * 128:(j + 1) * 128, :])
        fcb = cp.tile([1, NCLS], BF)
        nc.sync.dma_start(out=fcb, in_=dram("fcb"))

        # ---- persistent state ----
        hw_hist = cp.tile([128, 33 * 512], BF)   # h_t history, slot 0 = zeros
        nc.gpsimd.memset(hw_hist[:, 0:512], 0.0)
        hT0 = cp.tile([128, 512], BF)            # transposed h state, step -1
        nc.gpsimd.memset(hT0, 0.0)
        scores = cp.tile([128, 32], F32)
        ew = cp.tile([128, 32], F32)             # exp(scores)
        separts = cp.tile([128, 8], F32)         # partial exp sums (per 4-batch)
        # tiny dummy sigmoid: pulls the ACT_TABLE_LOAD for the sigmoid set
        # to kernel start, overlapping it with the weight DMAs
        nc.scalar.activation(separts[:, 0:1], ident[:, 0:1], AF.Sigmoid)
        sent = cp.tile([128, 512], BF)           # word-attention output
        sgi = cp.tile([128, 1536], BF)           # sentence-GRU input projections
        psgT = cp.tile([128, 12 * 128], BF)      # transposed gi: gate-chunk x rows
        hsf_hist = cp.tile([128, 17 * 32], BF)   # feature-major h^T history
        nc.gpsimd.memset(hsf_hist[:, 0:32], 0.0)
        hsb = cp.tile([128, 512], BF)            # sentence h, batch rows (s,d)
        hsbT = cp.tile([128, 4 * 128], BF)       # transposed: feat-chunk x rows

        # ================= word stage =================
        with tc.tile_pool(name="wp", bufs=3) as wp, \
             tc.tile_pool(name="wgi", bufs=5) as wgi, \
             tc.tile_pool(name="pp", bufs=1, space="PSUM") as pp:

            # PSUM banks (all resident for the whole loop):
            pga = pp.tile([128, 512], F32)   # rz pre-acts dir0
            pgb = pp.tile([128, 512], F32)   # rz pre-acts dir1
            pn = pp.tile([128, 512], F32)    # n pre-acts (both dirs)
            pu = pp.tile([128, 512], F32)    # attention u pre-acts
            pwa = pp.tile([128, 512], F32)   # resident exp-weighted h sum
            # transposes: two tiles padded to full banks so DVE reads of
            # pt0 can overlap PE writes of pt1 (no same-bank collision)
            pt0 = pp.tile([128, 256], BF, padded_shape=[128, 1024])
            pt1 = pp.tile([128, 256], BF, padded_shape=[128, 1024])

            PRE = 3  # gather prefetch depth
            gi_tiles = {}
            for t in range(PRE):
                g = wgi.tile([128, 1536], BF, tag="gi")
                nc.gpsimd.indirect_dma_start(
                    out=g[:, :], out_offset=None, in_=G_ap[:, :],
                    in_offset=bass.IndirectOffsetOnAxis(ap=toks[:, t:t + 1], axis=0),
                )
                gi_tiles[t] = g

            def inject(t):
                """Open PSUM accumulation groups for step t with gi + biases."""
                gi = gi_tiles[t]
                nc.tensor.matmul(pga, lhsT=ident, rhs=gi[:, 0:512],
                                 start=True, stop=False)
                nc.tensor.matmul(pgb, lhsT=ident, rhs=gi[:, 512:1024],
                                 start=True, stop=False)
                nc.tensor.matmul(pn, lhsT=ones, rhs=brow,
                                 start=True, stop=False)

            inject(0)

            prev_hT = hT0
            wsum_p = 0  # next pending weighted-sum step

            def wsum_step():
                """Accumulate one lagged exp-weighted h into resident pwa."""
                s = wsum_p
                dg = wp.tile([128, 128], BF, tag="dg")
                nc.vector.tensor_scalar_mul(dg, ident, ew[:, s:s + 1])
                nc.tensor.matmul(pwa, lhsT=dg,
                                 rhs=hw_hist[:, (s + 1) * 512:(s + 2) * 512],
                                 start=(s == 0), stop=(s == 31),
                                 skip_group_check=True)

            for t in range(32):
                gi = gi_tiles.pop(t)
                ds = (0, 1) if t % 2 == 0 else (1, 0)
                pg = {0: pga, 1: pgb}

                # --- PE: recurrent matmuls for step t (need prev_hT);
                #     the priority direction ds[0] alternates per step so
                #     neither direction's loop-carried chain always pays
                #     the other's queue delay ---
                for dd in ds:
                    for k in range(2):
                        lhs = prev_hT[:, (dd * 2 + k) * 128:(dd * 2 + k + 1) * 128]
                        w = whh[:, (dd * 2 + k) * GW:(dd * 2 + k + 1) * GW]
                        nc.tensor.matmul(pg[dd], lhsT=lhs, rhs=w[:, 0:512],
                                         start=False, stop=(k == 1))
                for dd in ds:
                    for k in range(2):
                        lhs = prev_hT[:, (dd * 2 + k) * 128:(dd * 2 + k + 1) * 128]
                        w = whh[:, (dd * 2 + k) * GW:(dd * 2 + k + 1) * GW]
                        nc.tensor.matmul(pn[:, dd * 256:(dd + 1) * 256],
                                         lhsT=lhs, rhs=w[:, 512:768],
                                         start=False,
                                         stop=(dd == ds[1] and k == 1))

                # --- ACT: sigmoids (start of per-dir gate chains) ---
                rz = wp.tile([128, 1024], BF, tag="rz")  # [r0 z0 | r1 z1]
                for dd in ds:
                    nc.scalar.activation(rz[:, dd * 512:(dd + 1) * 512],
                                         pg[dd], AF.Sigmoid)

                # --- PE: attention matmuls for step t-1 (fills stall) ---
                if t >= 1:
                    nc.tensor.matmul(pu, lhsT=ones, rhs=barow,
                                     start=True, stop=False)
                    for j in range(4):
                        nc.tensor.matmul(pu, lhsT=prev_hT[:, j * 128:(j + 1) * 128],
                                         rhs=waT[:, j * 512:(j + 1) * 512],
                                         start=False, stop=(j == 3))

                h_prev = hw_hist[:, t * 512:(t + 1) * 512]
                h_new = hw_hist[:, (t + 1) * 512:(t + 2) * 512]
                hT = wp.tile([128, 512], BF, tag="hT")
                nn = wp.tile([128, 512], BF, tag="nn")   # [n0 n1]
                t1_ = {0: wp.tile([128, 256], BF, tag="t1a", name="t1a"),
                       1: wp.tile([128, 256], BF, tag="t1b", name="t1b")}
                np_ = {0: wp.tile([128, 256], BF, tag="npa", name="npa"),
                       1: wp.tile([128, 256], BF, tag="npb", name="npb")}
                omz = wp.tile([128, 512], BF, tag="omz")
                zh = wp.tile([128, 512], BF, tag="zh")
                nom = {0: wp.tile([128, 256], BF, tag="noma", name="noma"),
                       1: wp.tile([128, 256], BF, tag="nomb", name="nomb")}

                def rslice(dd):
                    return rz[:, dd * 512:dd * 512 + 256]

                def zslice(dd):
                    return rz[:, dd * 512 + 256:(dd + 1) * 512]

                # --- gate chains: DVE does the serial path, gp the
                #     off-chain blend inputs, ACT the transcendentals ---
                # t1/np for both dirs first (they read pn, which the t+1
                # inject below overwrites -- program order defines deps)
                for dd in ds:
                    nc.vector.tensor_tensor(t1_[dd], rslice(dd),
                                            pn[:, dd * 256:(dd + 1) * 256],
                                            op=ALU.mult)
                    nc.vector.tensor_add(np_[dd], t1_[dd],
                                         gi[:, 1024 + dd * 256:
                                            1024 + (dd + 1) * 256])

                # gp: zh first (feeds the last chain op), then omz
                for dd in ds:
                    nc.gpsimd.tensor_tensor(zh[:, dd * 256:(dd + 1) * 256],
                                            zslice(dd),
                                            h_prev[:, dd * 256:(dd + 1) * 256],
                                            op=ALU.mult)
                for dd in ds:
                    nc.gpsimd.tensor_scalar(out=omz[:, dd * 256:(dd + 1) * 256],
                                            in0=zslice(dd), scalar1=-1.0,
                                            scalar2=1.0, op0=ALU.mult,
                                            op1=ALU.add)

                # --- PE: inject step t+1 (after sigmoid + t1 reads) ---
                if t < 31:
                    inject(t + 1)

                for dd in ds:
                    nc.scalar.activation(nn[:, dd * 256:(dd + 1) * 256],
                                         np_[dd], AF.Tanh)

                # --- per-dir tails + transposes; first dir's hT copy on
                #     DVE, second dir's on ACT ---
                for i, dd in enumerate(ds):
                    nc.vector.tensor_tensor(nom[dd], nn[:, dd * 256:(dd + 1) * 256],
                                            omz[:, dd * 256:(dd + 1) * 256],
                                            op=ALU.mult)
                    nc.vector.tensor_add(h_new[:, dd * 256:(dd + 1) * 256],
                                         nom[dd], zh[:, dd * 256:(dd + 1) * 256])
                    ptd = pt0 if dd == 0 else pt1
                    nc.tensor.transpose(ptd[:, 0:128],
                                        in_=h_new[:, dd * 256:dd * 256 + 128],
                                        identity=ident)
                    nc.tensor.transpose(ptd[:, 128:256],
                                        in_=h_new[:, dd * 256 + 128:(dd + 1) * 256],
                                        identity=ident)

                for i, dd in enumerate(ds):
                    ptd = pt0 if dd == 0 else pt1
                    if i == 0:
                        nc.vector.tensor_copy(hT[:, dd * 256:(dd + 1) * 256], ptd)
                    else:
                        nc.scalar.copy(hT[:, dd * 256:(dd + 1) * 256], ptd)
                prev_hT = hT

                # --- u(t-1) = tanh(pu); score via stt-accum ---
                if t >= 1:
                    u = wp.tile([128, 512], BF, tag="u")
                    nc.scalar.activation(u, pu, AF.Tanh)
                    scr = wp.tile([128, 512], BF, tag="scr")
                    nc.vector.scalar_tensor_tensor(
                        out=scr, in0=u, scalar=1.0, in1=vb,
                        op0=ALU.mult, op1=ALU.mult,
                        accum_out=scores[:, t - 1:t])

                # --- keep-warm dummy matmuls: fill the end-of-iteration
                # PE stall so HAM never sees an idle window (gated on
                # h_new so they land in the stall, and emitted after the
                # u-tanh read of pu so the garbage write is safe) ---
                if t >= 1:
                    nc.tensor.matmul(pu[:, 0:512], lhsT=h_new[:, 0:128],
                                     rhs=waT[:, 0:512], start=True, stop=True,
                                     skip_group_check=True)
                    nc.tensor.matmul(pu[:, 0:512], lhsT=h_new[:, 128:256],
                                     rhs=waT[:, 512:1024], start=True, stop=True,
                                     skip_group_check=True)
                    nc.tensor.matmul(pu[:, 0:512], lhsT=h_new[:, 256:384],
                                     rhs=waT[:, 1024:1536], start=True, stop=True,
                                     skip_group_check=True)
                    nc.tensor.matmul(pu[:, 0:512], lhsT=h_new[:, 384:512],
                                     rhs=waT[:, 1536:2048], start=True, stop=True,
                                     skip_group_check=True)

                # --- batched exp of scores, every 4 completed steps ---
                # exp(s) = 1/sigmoid(-s) - 1 (stays in the sigmoid table set)
                done = t
                if done % 4 == 0 and done > 0:
                    j = done // 4 - 1
                    sl = slice(j * 4, (j + 1) * 4)
                    nc.scalar.activation(separts[:, 0:4], scores[:, sl],
                                         AF.Sigmoid, scale=-1.0)
                    nc.vector.reciprocal(separts[:, 4:8], separts[:, 0:4])
                    nc.vector.tensor_scalar_add(ew[:, sl], separts[:, 4:8], -1.0)

                # --- lagged weighted-sum into resident pwa ---
                if wsum_p < (t // 4) * 4:
                    wsum_step()
                    wsum_p += 1

                # --- DMA: prefetch gather for step t+PRE ---
                if t + PRE < 32:
                    g = wgi.tile([128, 1536], BF, tag="gi")
                    nc.gpsimd.indirect_dma_start(
                        out=g[:, :], out_offset=None, in_=G_ap[:, :],
                        in_offset=bass.IndirectOffsetOnAxis(
                            ap=toks[:, t + PRE:t + PRE + 1], axis=0),
                    )
                    gi_tiles[t + PRE] = g

            # ---- word epilogue: attention for t=31 + remaining wsum ----
            nc.tensor.matmul(pu, lhsT=ones, rhs=barow, start=True, stop=False)
            for j in range(4):
                nc.tensor.matmul(pu, lhsT=prev_hT[:, j * 128:(j + 1) * 128],
                                 rhs=waT[:, j * 512:(j + 1) * 512],
                                 start=False, stop=(j == 3))
            u = wp.tile([128, 512], BF, tag="u")
            nc.scalar.activation(u, pu, AF.Tanh)
            scr = wp.tile([128, 512], BF, tag="scr")
            nc.vector.scalar_tensor_tensor(
                out=scr, in0=u, scalar=1.0, in1=vb,
                op0=ALU.mult, op1=ALU.mult, accum_out=scores[:, 31:32])
            nc.scalar.activation(separts[:, 0:4], scores[:, 28:32],
                                 AF.Sigmoid, scale=-1.0)
            nc.vector.reciprocal(separts[:, 4:8], separts[:, 0:4])
            nc.vector.tensor_scalar_add(ew[:, 28:32], separts[:, 4:8], -1.0)
            while wsum_p < 32:
                wsum_step()
                wsum_p += 1
            # normalize: sent = pwa / sum(exp)
            se = wp.tile([128, 1], F32, tag="se")
            nc.vector.tensor_reduce(se, ew, axis=mybir.AxisListType.X,
                                    op=ALU.add)
            rse = wp.tile([128, 1], F32, tag="rse")
            nc.vector.reciprocal(rse, se)
            nc.vector.tensor_scalar_mul(sent, pwa, rse)

        # ---- sent -> sentT + sentence input projections ----
        with tc.tile_pool(name="mid", bufs=1) as mp, \
             tc.tile_pool(name="pmid", bufs=1, space="PSUM") as pmp:
            ptm = pmp.tile([128, 512], BF, tag="ptm")
            for j in range(4):
                nc.tensor.transpose(ptm[:, j * 128:(j + 1) * 128],
                                    in_=sent[:, j * 128:(j + 1) * 128],
                                    identity=ident)
            sentT = mp.tile([128, 512], BF)
            nc.vector.tensor_copy(sentT[:, 0:256], ptm[:, 0:256])
            nc.scalar.copy(sentT[:, 256:512], ptm[:, 256:512])

            psg = pmp.tile([128, 1536], F32, tag="psg")
            for ns in range(3):
                sl = slice(ns * 512, (ns + 1) * 512)
                nc.tensor.matmul(psg[:, sl], lhsT=ones, rhs=sprow[:, sl],
                                 start=True, stop=False)
                for k in range(4):
                    nc.tensor.matmul(psg[:, sl],
                                     lhsT=sentT[:, k * 128:(k + 1) * 128],
                                     rhs=swihT[:, k * 1536 + ns * 512:
                                               k * 1536 + (ns + 1) * 512],
                                     start=False, stop=(k == 3))
            nc.scalar.copy(sgi[:, 0:512], psg[:, 0:512])
            nc.scalar.copy(sgi[:, 512:1024], psg[:, 512:1024])
            nc.vector.tensor_copy(sgi[:, 1024:1536], psg[:, 1024:1536])
            # transpose sgi into feature-major gate-chunk slots, grouped
            # per direction:
            # [d0: r0 r1 z0 z1 | d1: r0 r1 z0 z1 | d0: n0 n1 | d1: n0 n1]
            slot_src = [0, 128, 256, 384, 512, 640, 768, 896,
                        1024, 1152, 1280, 1408]
            ptg = pmp.tile([128, 1536], BF, tag="ptg")
            for j, src in enumerate(slot_src):
                nc.tensor.transpose(ptg[:, j * 128:(j + 1) * 128],
                                    in_=sgi[:, src:src + 128],
                                    identity=ident)
            psgT3c = psgT.rearrange("p (g r) -> p g r", g=12)
            ptg3c = ptg.rearrange("p (g r) -> p g r", g=12)
            nc.scalar.copy(psgT3c[:, :, 0:64], ptg3c[:, :, 0:64])
            nc.vector.tensor_copy(psgT3c[:, :, 64:128], ptg3c[:, :, 64:128])

        # ================= sentence stage =================
        # Feature-major sentence GRU: state h^T [128 feat(chunk d,m), 8 docs],
        # weight-stationary rec matmuls (N=8), gate math at FD<=32,
        # directions staggered so d1's matmuls stream during d0's gate math.
        with tc.tile_pool(name="sp", bufs=3) as sp:
          with tc.tile_pool(name="pgs", bufs=2, space="PSUM") as pgsp:
            psgT3 = psgT.rearrange("p (g r) -> p g r", g=12)

            for t in range(16):
                hprev = hsf_hist[:, t * 32:(t + 1) * 32]
                h_new = hsf_hist[:, (t + 1) * 32:(t + 2) * 32]
                prz_full = pgsp.tile([128, 512], F32, tag="prz")
                pnn_full = pgsp.tile([128, 512], F32, tag="pnn")
                prz = prz_full[:, 0:64]   # [d0: r0 r1 z0 z1 | d1: ...] x 8
                pnn = pnn_full[:, 0:32]   # [d0: n0 n1 | d1: n0 n1] x 8

                for d in ((0, 1) if t % 2 == 0 else (1, 0)):
                    # --- PE: bias opener + recurrent matmuls, dir d ---
                    nc.tensor.matmul(pnn[:, d * 16:(d + 1) * 16], lhsT=sbrowF,
                                     rhs=bones[:, d * 16:(d + 1) * 16],
                                     start=True, stop=False)
                    for sl_ in range(4):  # (g in r,z) x (m in 0,1)
                        for k in range(2):
                            w_i = d * 8 + sl_ * 2 + k
                            w = swhhF[:, w_i * 128:(w_i + 1) * 128]
                            nc.tensor.matmul(
                                prz[:, d * 32 + sl_ * 8:d * 32 + (sl_ + 1) * 8],
                                lhsT=w,
                                rhs=hprev[:, (d * 2 + k) * 8:(d * 2 + k + 1) * 8],
                                start=(k == 0), stop=(k == 1))
                    for m in range(2):
                        for k in range(2):
                            w_i = 16 + d * 4 + m * 2 + k
                            w = swhhF[:, w_i * 128:(w_i + 1) * 128]
                            nc.tensor.matmul(
                                pnn[:, d * 16 + m * 8:d * 16 + (m + 1) * 8],
                                lhsT=w,
                                rhs=hprev[:, (d * 2 + k) * 8:(d * 2 + k + 1) * 8],
                                start=False, stop=(m == 1 and k == 1))

                    # --- gate math, dir d (tiny FD; runs while other dir's
                    #     matmuls stream on PE) ---
                    przd = prz[:, d * 32:(d + 1) * 32]
                    pnnd = pnn[:, d * 16:(d + 1) * 16]
                    hprevd = hprev[:, d * 16:(d + 1) * 16]
                    hnewd = h_new[:, d * 16:(d + 1) * 16]
                    rzp = sp.tile([128, 32], BF, tag=f"srzp{d}")
                    nc.vector.tensor_tensor(
                        rzp.rearrange("p (g r) -> p g r", g=4),
                        przd.rearrange("p (g r) -> p g r", g=4),
                        psgT3[:, d * 4:(d + 1) * 4, t * 8:(t + 1) * 8],
                        op=ALU.add)
                    rzd = sp.tile([128, 32], BF, tag=f"srz{d}")
                    nc.scalar.activation(rzd, rzp, AF.Sigmoid)
                    t1 = sp.tile([128, 16], BF, tag=f"st1{d}")
                    nc.vector.tensor_tensor(t1, rzd[:, 0:16], pnnd, op=ALU.mult)
                    npre = sp.tile([128, 16], BF, tag=f"snp{d}")
                    nc.vector.tensor_tensor(
                        npre.rearrange("p (g r) -> p g r", g=2),
                        t1.rearrange("p (g r) -> p g r", g=2),
                        psgT3[:, 8 + d * 2:8 + (d + 1) * 2, t * 8:(t + 1) * 8],
                        op=ALU.add)
                    omz = sp.tile([128, 16], BF, tag=f"som{d}")
                    nc.gpsimd.tensor_scalar(out=omz, in0=rzd[:, 16:32],
                                            scalar1=-1.0, scalar2=1.0,
                                            op0=ALU.mult, op1=ALU.add)
                    zh = sp.tile([128, 16], BF, tag=f"szh{d}")
                    nc.gpsimd.tensor_tensor(zh, rzd[:, 16:32], hprevd,
                                            op=ALU.mult)
                    nnT = sp.tile([128, 16], BF, tag=f"snn{d}")
                    nc.scalar.activation(nnT, npre, AF.Tanh)
                    nom = sp.tile([128, 16], BF, tag=f"snm{d}")
                    nc.vector.tensor_tensor(nom, nnT, omz, op=ALU.mult)
                    nc.vector.tensor_add(hnewd, nom, zh)

                    # batch layout for the attention post-pass
                    hsbT4 = hsbT.rearrange("p (d c r) -> p d c r", d=2, c=2)
                    nc.scalar.copy(
                        hsbT4[:, d, :, 8 * t:8 * (t + 1)],
                        hnewd.rearrange("p (c r) -> p c r", c=2))

            # rebuild batch-major hsb from hsbT (4 transposes)
            pth = pgsp.tile([128, 512], BF, tag="pth")
            for j in range(4):
                nc.tensor.transpose(pth[:, j * 128:(j + 1) * 128],
                                    in_=hsbT[:, j * 128:(j + 1) * 128],
                                    identity=ident)
            nc.vector.tensor_copy(hsb[:, 0:256], pth[:, 0:256])
            nc.scalar.copy(hsb[:, 256:512], pth[:, 256:512])

          # ---- sentence attention post-pass (batch rows = (s,d)) ----
          with tc.tile_pool(name="pps", bufs=1, space="PSUM") as ppsp:
            pu2 = ppsp.tile([128, 512], F32, tag="pu2")
            nc.tensor.matmul(pu2, lhsT=ones, rhs=sbarow, start=True, stop=False)
            for j in range(4):
                nc.tensor.matmul(pu2, lhsT=hsbT[:, j * 128:(j + 1) * 128],
                                 rhs=sawT[:, j * 512:(j + 1) * 512],
                                 start=False, stop=(j == 3))
            u2 = sp.tile([128, 512], BF, tag="u2")
            nc.scalar.activation(u2, pu2, AF.Tanh)
            s_sc = sp.tile([128, 1], F32, tag="s_sc")
            scr2 = sp.tile([128, 512], BF, tag="scr2")
            nc.vector.scalar_tensor_tensor(
                out=scr2, in0=u2, scalar=1.0, in1=svb,
                op0=ALU.mult, op1=ALU.mult, accum_out=s_sc)
            # exp via sigmoid set: exp(s) = 1/sigmoid(-s) - 1
            ew2s = sp.tile([128, 1], F32, tag="ew2s")
            nc.scalar.activation(ew2s, s_sc, AF.Sigmoid, scale=-1.0)
            ew2r = sp.tile([128, 1], F32, tag="ew2r")
            nc.vector.reciprocal(ew2r, ew2s)
            ew2 = sp.tile([128, 1], F32, tag="ew2")
            nc.vector.tensor_scalar_add(ew2, ew2r, -1.0)
            # per-doc sums via indicator matmul, then broadcast back
            pds = ppsp.tile([8, 1], F32, tag="pds")
            nc.tensor.matmul(pds, lhsT=ind8f, rhs=ew2, start=True, stop=True)
            dsum = sp.tile([8, 1], F32, tag="dsum")
            nc.vector.tensor_copy(dsum, pds)
            rds = sp.tile([8, 1], F32, tag="rds")
            nc.vector.reciprocal(rds, dsum)
            prb = ppsp.tile([128, 1], F32, tag="prb")
            nc.tensor.matmul(prb, lhsT=ind8T, rhs=rds, start=True, stop=True)
            aw = sp.tile([128, 1], F32, tag="aw")
            nc.vector.tensor_tensor(aw, ew2, prb, op=ALU.mult)
            awd = sp.tile([128, 8], BF, tag="awd")
            nc.vector.tensor_scalar_mul(awd, ind8, aw)
            pdoc = ppsp.tile([8, 512], F32, tag="pdoc")
            nc.tensor.matmul(pdoc, lhsT=awd, rhs=hsb, start=True, stop=True)
            doc = sp.tile([8, 512], BF, tag="doc")
            nc.scalar.copy(doc, pdoc)

            # ---- classifier + log_softmax ----
            ptd = ppsp.tile([128, 32], BF, tag="ptd")
            for j in range(4):
                nc.tensor.transpose(ptd[:, j * 8:(j + 1) * 8],
                                    in_=doc[:, j * 128:(j + 1) * 128],
                                    identity=ident[0:8, 0:8])
            docT = sp.tile([128, 32], BF, tag="docT")
            nc.vector.tensor_copy(docT, ptd)
            pl = ppsp.tile([8, NCLS], F32, tag="pl")
            nc.tensor.matmul(pl, lhsT=ones[:, 0:8], rhs=fcb,
                             start=True, stop=False)
            for j in range(4):
                nc.tensor.matmul(pl, lhsT=docT[:, j * 8:(j + 1) * 8],
                                 rhs=fcwT[:, j * NCLS:(j + 1) * NCLS],
                                 start=False, stop=(j == 3))
            nmx2 = sp.tile([8, 1], F32, tag="nmx2")
            nc.vector.tensor_reduce(nmx2, pl, axis=mybir.AxisListType.X,
                                    op=ALU.max, negate=True)
            e2 = sp.tile([8, NCLS], F32, tag="e2")
            se2 = sp.tile([8, 1], F32, tag="se2")
            nc.scalar.activation(e2, pl, AF.Exp, bias=nmx2, accum_out=se2)
            lse = sp.tile([8, 1], F32, tag="lse")
            nc.scalar.activation(lse, se2, AF.Ln)
            out_sb = sp.tile([8, NCLS], F32, tag="out_sb")
            nc.vector.tensor_scalar(out=out_sb, in0=pl, scalar1=nmx2,
                                    scalar2=lse, op0=ALU.add, op1=ALU.subtract)
            nc.sync.dma_start(out=dram("out"), in_=out_sb)


# ---------------------------------------------------------------------------
# host side
# ---------------------------------------------------------------------------

def _prep_inputs(inputs):
    """Build the per-core in_maps (host preprocessing + sharding)."""
    f32 = np.float32
    emb = np.asarray(inputs["emb"], f32)
    w_Wih = np.asarray(inputs["w_Wih"], f32)
    w_Whh = np.asarray(inputs["w_Whh"], f32)
    w_bih = np.asarray(inputs["w_bih"], f32)
    w_bhh = np.asarray(inputs["w_bhh"], f32)
    wa_W = np.asarray(inputs["wa_W"], f32)
    wa_b = np.asarray(inputs["wa_b"], f32)
    wa_v = np.asarray(inputs["wa_v"], f32)
    s_Wih = np.asarray(inputs["s_Wih"], f32)
    s_Whh = np.asarray(inputs["s_Whh"], f32)
    s_bih = np.asarray(inputs["s_bih"], f32)
    s_bhh = np.asarray(inputs["s_bhh"], f32)
    sa_W = np.asarray(inputs["sa_W"], f32)
    sa_b = np.asarray(inputs["sa_b"], f32)
    sa_v = np.asarray(inputs["sa_v"], f32)
    fc_W = np.asarray(inputs["fc_W"], f32)
    fc_b = np.asarray(inputs["fc_b"], f32)
    tokens = np.asarray(inputs["tokens"])

    def b(x):
        return np.ascontiguousarray(x.astype(bf16))

    # folded gather table G [V, 1536] = [rz0 | rz1 | n0 | n1]
    g0 = emb @ w_Wih[0].T + w_bih[0]
    g0[:, :512] += w_bhh[0][:512]
    g1 = emb @ w_Wih[1].T + w_bih[1]
    g1[:, :512] += w_bhh[1][:512]
    G = np.concatenate([g0[:, :512], g1[:, :512], g0[:, 512:], g1[:, 512:]], 1)

    whhT = np.stack([w_Whh[0].T[:128], w_Whh[0].T[128:],
                     w_Whh[1].T[:128], w_Whh[1].T[128:]])  # [4,128,768]
    brow = np.concatenate([w_bhh[0][512:], w_bhh[1][512:]])[None, :]
    vbc = np.broadcast_to(wa_v, (128, 512))

    # sentence input-proj table [512, 1536] with same col layout; bias row
    sg0 = s_Wih[0].T  # [512, 768]
    sg1 = s_Wih[1].T
    swihT = np.concatenate([sg0[:, :512], sg1[:, :512],
                            sg0[:, 512:], sg1[:, 512:]], 1)
    sprow = np.concatenate([
        s_bih[0][:512] + s_bhh[0][:512],
        s_bih[1][:512] + s_bhh[1][:512],
        s_bih[0][512:], s_bih[1][512:]])[None, :]
    # feature-major weight chunks for the sentence GRU, slot order grouped
    # by direction: rz: for d: for g in (r,z): for m: for k;
    # n: for d: for m: for k
    goff = {"r": 0, "z": 256, "n": 512}
    fch = []
    for d in range(2):
        for g in ("r", "z"):
            for m in range(2):
                for k in range(2):
                    fch.append(s_Whh[d][goff[g] + m * 128:goff[g] + (m + 1) * 128,
                                        k * 128:(k + 1) * 128].T)
    for d in range(2):
        for m in range(2):
            for k in range(2):
                fch.append(s_Whh[d][goff["n"] + m * 128:goff["n"] + (m + 1) * 128,
                                    k * 128:(k + 1) * 128].T)
    swhhF = np.stack(fch)  # [24, 128, 128]
    sbrow = np.concatenate([s_bhh[0][512:], s_bhh[1][512:]])[None, :]
    svbc = np.broadcast_to(sa_v, (128, 512))

    ind = np.zeros((128, 8), f32)
    for row in range(128):
        ind[row, row % 8] = 1.0

    shared = {
        "G": b(G), "whhT": b(whhT), "brow": b(brow),
        "waT": b(wa_W.T), "barow": b(wa_b[None, :]), "vb": b(vbc),
        "swihT": b(swihT), "sprow": b(sprow),
        "swhhF": b(swhhF),
        "sbrowF": b(sbrow.reshape(4, 128)),
        "bones": b(np.repeat(np.eye(4, dtype=f32), 8, axis=1)),
        "sbrow": b(sbrow), "sawT": b(sa_W.T), "sbarow": b(sa_b[None, :]),
        "svb": b(svbc), "fcwT": b(fc_W.T), "fcb": b(fc_b[None, :]),
        "ind8": b(ind), "ind8f": np.ascontiguousarray(ind),
        "ind8T": np.ascontiguousarray(ind.T),
    }
    in_maps = []
    for c in range(NCORES):
        # word-row p = s*8 + doc  (so sentence step s owns partition rows
        # [s*8:(s+1)*8] of the batch-major sentence matrix)
        tk = np.ascontiguousarray(
            np.transpose(tokens[c * BC:(c + 1) * BC], (1, 0, 2))
            .reshape(NW, W).astype(np.int32))
        in_maps.append({**shared, "toks": tk})
    return in_maps


_NC_CACHE = {}


def _get_nc():
    if "nc" not in _NC_CACHE:
        _NC_CACHE["nc"] = _build_program()
    return _NC_CACHE["nc"]


def kernel(**inputs) -> np.ndarray:
    nc = _get_nc()
    in_maps = _prep_inputs(inputs)
    res = bass_utils.run_bass_kernel_spmd(nc, in_maps, core_ids=list(range(NCORES)))
    outs = []
    for c in range(NCORES):
        o = np.asarray(res.results[c]["out"], np.float32)
        outs.append(o)
    return np.concatenate(outs, 0)


# revision 19
# speedup vs baseline: 1.3151x; 1.3151x over previous
"""HAN (hierarchical attention network) forward pass on 8 TRN2 NeuronCores.

Strategy
--------
Data-parallel over batch: each core handles 8 documents = 128 sentences =
4096 tokens, fully independently (no collectives). Inside a core:

* Embedding lookup + word-GRU input projection folded on host:
  gi = (emb @ Wih.T)[tokens], gathered per step with indirect DMA.
* Word bi-GRU, batch-major [128 sentences, feat]. Per-direction gate math
  so the two independent direction recurrences pipeline across ACT/DVE/
  GPSIMD. Engine programs are ordered so the d0 chain (rec matmul ->
  sigmoid -> r*hn -> +gi -> tanh -> blend -> transpose -> copy) never
  waits behind low-priority work; attention and injects fill PE gaps.
* gi injection PSUM groups are single-buffered (inject for t+1 reuses the
  banks right after sigmoid(t) reads them), freeing banks for TWO
  transpose banks (pt0/pt1) so hT copies overlap the second transpose
  pair, and for a RESIDENT weighted-sum accumulator bank (diag(exp(s))
  matmuls accumulate across all 32 steps; no per-4-step PSUM drain).
* Word attention: scores accumulated in-loop (u = tanh(h Wa + ba) lagged
  one step; u.v via stt-accum). Softmax without max-subtraction as an
  exp-weighted running matmul sum.
* Sentence bi-GRU: feature-major (free dims 16-32), the two directions
  emitted staggered: d1's 12 weight-load+matmul pairs stream on PE while
  d0's gate math runs on ACT/DVE, and vice versa. Sentence attention is
  a post-pass batch GEMM; per-document softmax via indicator matmuls.

Compute dtype bf16 (fp32 PSUM accumulation + fp32 attention
accumulators).
"""

import numpy as np
import ml_dtypes

import concourse.bass as bass
import concourse.mybir as mybir
import concourse.tile as tile
from concourse import bacc, bass_utils
from concourse.masks import make_identity

BF = mybir.dt.bfloat16
F32 = mybir.dt.float32
AF = mybir.ActivationFunctionType
ALU = mybir.AluOpType
bf16 = ml_dtypes.bfloat16

V, E = 50000, 300
HW_, HS_ = 256, 256
NCLS = 10
B, S, W = 64, 16, 32
NCORES = 8
BC = B // NCORES          # docs per core = 8
NW = BC * S               # word-level batch per core = 128
GW = 3 * HW_              # 768


def _build_program():
    nc = bacc.Bacc(
        "TRN2",
        target_bir_lowering=False,
        debug=False,
        enable_asserts=False,
        num_devices=NCORES,
    )

    # ---- DRAM I/O ----
    h = {}
    h["G"] = nc.dram_tensor("G", [V, 1536], BF, kind="ExternalInput")
    h["toks"] = nc.dram_tensor("toks", [128, 32], mybir.dt.int32, kind="ExternalInput")
    h["whhT"] = nc.dram_tensor("whhT", [4, 128, GW], BF, kind="ExternalInput")
    h["brow"] = nc.dram_tensor("brow", [1, 512], BF, kind="ExternalInput")
    h["waT"] = nc.dram_tensor("waT", [512, 512], BF, kind="ExternalInput")
    h["barow"] = nc.dram_tensor("barow", [1, 512], BF, kind="ExternalInput")
    h["vb"] = nc.dram_tensor("vb", [128, 512], BF, kind="ExternalInput")
    h["swihT"] = nc.dram_tensor("swihT", [512, 1536], BF, kind="ExternalInput")
    h["sprow"] = nc.dram_tensor("sprow", [1, 1536], BF, kind="ExternalInput")
    h["swhhF"] = nc.dram_tensor("swhhF", [24, 128, 128], BF, kind="ExternalInput")
    h["sbrowF"] = nc.dram_tensor("sbrowF", [4, 128], BF, kind="ExternalInput")
    h["bones"] = nc.dram_tensor("bones", [4, 32], BF, kind="ExternalInput")
    h["sbrow"] = nc.dram_tensor("sbrow", [1, 512], BF, kind="ExternalInput")
    h["sawT"] = nc.dram_tensor("sawT", [512, 512], BF, kind="ExternalInput")
    h["sbarow"] = nc.dram_tensor("sbarow", [1, 512], BF, kind="ExternalInput")
    h["svb"] = nc.dram_tensor("svb", [128, 512], BF, kind="ExternalInput")
    h["ind8"] = nc.dram_tensor("ind8", [128, 8], BF, kind="ExternalInput")
    h["ind8f"] = nc.dram_tensor("ind8f", [128, 8], F32, kind="ExternalInput")
    h["ind8T"] = nc.dram_tensor("ind8T", [8, 128], F32, kind="ExternalInput")
    h["fcwT"] = nc.dram_tensor("fcwT", [512, NCLS], BF, kind="ExternalInput")
    h["fcb"] = nc.dram_tensor("fcb", [1, NCLS], BF, kind="ExternalInput")
    h["out"] = nc.dram_tensor("out", [BC, NCLS], F32, kind="ExternalOutput")

    with tile.TileContext(nc) as tc:
        _body(nc, tc, h)
    nc.compile()
    return nc


def _body(nc, tc, handles):
    def dram(name):
        return handles[name].ap()

    G_ap = dram("G")
    with tc.tile_pool(name="const", bufs=1) as cp:
        # ---- constants / weights in SBUF ----
        ident = cp.tile([128, 128], BF)
        make_identity(nc, ident)
        ones = cp.tile([1, 128], BF)
        nc.gpsimd.memset(ones, 1.0)

        toks = cp.tile([128, 32], mybir.dt.int32)
        nc.sync.dma_start(out=toks, in_=dram("toks"))
        whh = cp.tile([128, 4 * GW], BF)  # (d0k0 d0k1 d1k0 d1k1); [rz(512) n(256)]
        for j in range(4):
            nc.sync.dma_start(out=whh[:, j * GW:(j + 1) * GW],
                              in_=dram("whhT")[j])
        brow = cp.tile([1, 512], BF)
        nc.sync.dma_start(out=brow, in_=dram("brow"))
        waT = cp.tile([128, 4 * 512], BF)
        for j in range(4):
            nc.sync.dma_start(out=waT[:, j * 512:(j + 1) * 512],
                              in_=dram("waT")[j * 128:(j + 1) * 128, :])
        barow = cp.tile([1, 512], BF)
        nc.sync.dma_start(out=barow, in_=dram("barow"))
        vb = cp.tile([128, 512], BF)
        nc.sync.dma_start(out=vb, in_=dram("vb"))

        swihT = cp.tile([128, 4 * 1536], BF)
        for j in range(4):
            nc.sync.dma_start(out=swihT[:, j * 1536:(j + 1) * 1536],
                              in_=dram("swihT")[j * 128:(j + 1) * 128, :])
        sprow = cp.tile([1, 1536], BF)
        nc.sync.dma_start(out=sprow, in_=dram("sprow"))
        swhhF = cp.tile([128, 24 * 128], BF)
        for j in range(24):
            nc.sync.dma_start(out=swhhF[:, j * 128:(j + 1) * 128],
                              in_=dram("swhhF")[j])
        sbrow = cp.tile([1, 512], BF)
        nc.sync.dma_start(out=sbrow, in_=dram("sbrow"))
        sbrowF = cp.tile([4, 128], BF)
        nc.sync.dma_start(out=sbrowF, in_=dram("sbrowF"))
        bones = cp.tile([4, 32], BF)
        nc.sync.dma_start(out=bones, in_=dram("bones"))
        sawT = cp.tile([128, 4 * 512], BF)
        for j in range(4):
            nc.sync.dma_start(out=sawT[:, j * 512:(j + 1) * 512],
                              in_=dram("sawT")[j * 128:(j + 1) * 128, :])
        sbarow = cp.tile([1, 512], BF)
        nc.sync.dma_start(out=sbarow, in_=dram("sbarow"))
        svb = cp.tile([128, 512], BF)
        nc.sync.dma_start(out=svb, in_=dram("svb"))
        ind8 = cp.tile([128, 8], BF)
        nc.sync.dma_start(out=ind8, in_=dram("ind8"))
        ind8f = cp.tile([128, 8], F32)
        nc.sync.dma_start(out=ind8f, in_=dram("ind8f"))
        ind8T = cp.tile([8, 128], F32)
        nc.sync.dma_start(out=ind8T, in_=dram("ind8T"))
        fcwT = cp.tile([128, 4 * NCLS], BF)
        for j in range(4):
            nc.sync.dma_start(out=fcwT[:, j * NCLS:(j + 1) * NCLS],
                              in_=dram("fcwT")[j * 128:(j + 1) * 128, :])
        fcb = cp.tile([1, NCLS], BF)
        nc.sync.dma_start(out=fcb, in_=dram("fcb"))

        # ---- persistent state ----
        hw_hist = cp.tile([128, 33 * 512], BF)   # h_t history, slot 0 = zeros
        nc.gpsimd.memset(hw_hist[:, 0:512], 0.0)
        hT0 = cp.tile([128, 512], BF)            # transposed h state, step -1
        nc.gpsimd.memset(hT0, 0.0)
        scores = cp.tile([128, 32], F32)
        ew = cp.tile([128, 32], F32)             # exp(scores)
        separts = cp.tile([128, 8], F32)         # partial exp sums (per 4-batch)
        # tiny dummy sigmoid: pulls the ACT_TABLE_LOAD for the sigmoid set
        # to kernel start, overlapping it with the weight DMAs
        nc.scalar.activation(separts[:, 0:1], ident[:, 0:1], AF.Sigmoid)
        sent = cp.tile([128, 512], BF)           # word-attention output
        sgi = cp.tile([128, 1536], BF)           # sentence-GRU input projections
        psgT = cp.tile([128, 12 * 128], BF)      # transposed gi: gate-chunk x rows
        hsf_hist = cp.tile([128, 17 * 32], BF)   # feature-major h^T history
        nc.gpsimd.memset(hsf_hist[:, 0:32], 0.0)
        hsb = cp.tile([128, 512], BF)            # sentence h, batch rows (s,d)
        hsbT = cp.tile([128, 4 * 128], BF)       # transposed: feat-chunk x rows

        # ================= word stage =================
        with tc.tile_pool(name="wp", bufs=3) as wp, \
             tc.tile_pool(name="wgi", bufs=5) as wgi, \
             tc.tile_pool(name="pp", bufs=1, space="PSUM") as pp:

            # PSUM banks (all resident for the whole loop):
            pga = pp.tile([128, 512], F32)   # rz pre-acts dir0
            pgb = pp.tile([128, 512], F32)   # rz pre-acts dir1
            pn = pp.tile([128, 512], F32)    # n pre-acts (both dirs)
            pu = pp.tile([128, 512], F32)    # attention u pre-acts
            pwa = pp.tile([128, 512], F32)   # resident exp-weighted h sum
            # transposes: two tiles padded to full banks so DVE reads of
            # pt0 can overlap PE writes of pt1 (no same-bank collision)
            pt0 = pp.tile([128, 256], BF, padded_shape=[128, 1024])
            pt1 = pp.tile([128, 256], BF, padded_shape=[128, 1024])

            PRE = 3  # gather prefetch depth
            gi_tiles = {}
            for t in range(PRE):
                g = wgi.tile([128, 1536], BF, tag="gi")
                nc.gpsimd.indirect_dma_start(
                    out=g[:, :], out_offset=None, in_=G_ap[:, :],
                    in_offset=bass.IndirectOffsetOnAxis(ap=toks[:, t:t + 1], axis=0),
                )
                gi_tiles[t] = g

            def inject(t):
                """Open PSUM accumulation groups for step t with gi + biases."""
                gi = gi_tiles[t]
                nc.tensor.matmul(pga, lhsT=ident, rhs=gi[:, 0:512],
                                 start=True, stop=False)
                nc.tensor.matmul(pgb, lhsT=ident, rhs=gi[:, 512:1024],
                                 start=True, stop=False)
                nc.tensor.matmul(pn, lhsT=ones, rhs=brow,
                                 start=True, stop=False)

            inject(0)

            prev_hT = hT0
            wsum_p = 0  # next pending weighted-sum step

            def wsum_step():
                """Accumulate one lagged exp-weighted h into resident pwa."""
                s = wsum_p
                dg = wp.tile([128, 128], BF, tag="dg")
                nc.vector.tensor_scalar_mul(dg, ident, ew[:, s:s + 1])
                nc.tensor.matmul(pwa, lhsT=dg,
                                 rhs=hw_hist[:, (s + 1) * 512:(s + 2) * 512],
                                 start=(s == 0), stop=(s == 31),
                                 skip_group_check=True)

            for t in range(32):
                gi = gi_tiles.pop(t)
                ds = (0, 1)
                pg = {0: pga, 1: pgb}

                # --- PE: recurrent matmuls for step t (need prev_hT) ---
                for dd in ds:
                    for k in range(2):
                        lhs = prev_hT[:, (dd * 2 + k) * 128:(dd * 2 + k + 1) * 128]
                        w = whh[:, (dd * 2 + k) * GW:(dd * 2 + k + 1) * GW]
                        nc.tensor.matmul(pg[dd], lhsT=lhs, rhs=w[:, 0:512],
                                         start=False, stop=(k == 1))
                for dd in ds:
                    for k in range(2):
                        lhs = prev_hT[:, (dd * 2 + k) * 128:(dd * 2 + k + 1) * 128]
                        w = whh[:, (dd * 2 + k) * GW:(dd * 2 + k + 1) * GW]
                        nc.tensor.matmul(pn[:, dd * 256:(dd + 1) * 256],
                                         lhsT=lhs, rhs=w[:, 512:768],
                                         start=False,
                                         stop=(dd == ds[1] and k == 1))

                # --- ACT: sigmoids (start of per-dir gate chains) ---
                rz = wp.tile([128, 1024], BF, tag="rz")  # [r0 z0 | r1 z1]
                for dd in ds:
                    nc.scalar.activation(rz[:, dd * 512:(dd + 1) * 512],
                                         pg[dd], AF.Sigmoid)

                # --- PE: attention matmuls for step t-1 (fills stall) ---
                if t >= 1:
                    nc.tensor.matmul(pu, lhsT=ones, rhs=barow,
                                     start=True, stop=False)
                    for j in range(4):
                        nc.tensor.matmul(pu, lhsT=prev_hT[:, j * 128:(j + 1) * 128],
                                         rhs=waT[:, j * 512:(j + 1) * 512],
                                         start=False, stop=(j == 3))

                h_prev = hw_hist[:, t * 512:(t + 1) * 512]
                h_new = hw_hist[:, (t + 1) * 512:(t + 2) * 512]
                hT = wp.tile([128, 512], BF, tag="hT")
                nn = wp.tile([128, 512], BF, tag="nn")   # [n0 n1]
                t1_ = {0: wp.tile([128, 256], BF, tag="t1a", name="t1a"),
                       1: wp.tile([128, 256], BF, tag="t1b", name="t1b")}
                np_ = {0: wp.tile([128, 256], BF, tag="npa", name="npa"),
                       1: wp.tile([128, 256], BF, tag="npb", name="npb")}
                omz = wp.tile([128, 512], BF, tag="omz")
                zh = wp.tile([128, 512], BF, tag="zh")
                nom = {0: wp.tile([128, 256], BF, tag="noma", name="noma"),
                       1: wp.tile([128, 256], BF, tag="nomb", name="nomb")}

                def rslice(dd):
                    return rz[:, dd * 512:dd * 512 + 256]

                def zslice(dd):
                    return rz[:, dd * 512 + 256:(dd + 1) * 512]

                # --- gate chains: DVE does the serial path, gp the
                #     off-chain blend inputs, ACT the transcendentals ---
                # t1/np for both dirs first (they read pn, which the t+1
                # inject below overwrites -- program order defines deps)
                for dd in ds:
                    nc.vector.tensor_tensor(t1_[dd], rslice(dd),
                                            pn[:, dd * 256:(dd + 1) * 256],
                                            op=ALU.mult)
                    nc.vector.tensor_add(np_[dd], t1_[dd],
                                         gi[:, 1024 + dd * 256:
                                            1024 + (dd + 1) * 256])

                # gp: zh first (feeds the last chain op), then omz
                for dd in ds:
                    nc.gpsimd.tensor_tensor(zh[:, dd * 256:(dd + 1) * 256],
                                            zslice(dd),
                                            h_prev[:, dd * 256:(dd + 1) * 256],
                                            op=ALU.mult)
                for dd in ds:
                    nc.gpsimd.tensor_scalar(out=omz[:, dd * 256:(dd + 1) * 256],
                                            in0=zslice(dd), scalar1=-1.0,
                                            scalar2=1.0, op0=ALU.mult,
                                            op1=ALU.add)

                # --- PE: inject step t+1 (after sigmoid + t1 reads) ---
                if t < 31:
                    inject(t + 1)

                for dd in ds:
                    nc.scalar.activation(nn[:, dd * 256:(dd + 1) * 256],
                                         np_[dd], AF.Tanh)

                # --- per-dir tails + transposes; first dir's hT copy on
                #     DVE, second dir's on ACT ---
                for i, dd in enumerate(ds):
                    nc.vector.tensor_tensor(nom[dd], nn[:, dd * 256:(dd + 1) * 256],
                                            omz[:, dd * 256:(dd + 1) * 256],
                                            op=ALU.mult)
                    nc.vector.tensor_add(h_new[:, dd * 256:(dd + 1) * 256],
                                         nom[dd], zh[:, dd * 256:(dd + 1) * 256])
                    ptd = pt0 if dd == 0 else pt1
                    nc.tensor.transpose(ptd[:, 0:128],
                                        in_=h_new[:, dd * 256:dd * 256 + 128],
                                        identity=ident)
                    nc.tensor.transpose(ptd[:, 128:256],
                                        in_=h_new[:, dd * 256 + 128:(dd + 1) * 256],
                                        identity=ident)

                for i, dd in enumerate(ds):
                    ptd = pt0 if dd == 0 else pt1
                    if i == 0:
                        nc.vector.tensor_copy(hT[:, dd * 256:(dd + 1) * 256], ptd)
                    else:
                        nc.scalar.copy(hT[:, dd * 256:(dd + 1) * 256], ptd)
                prev_hT = hT

                # --- u(t-1) = tanh(pu); score via stt-accum ---
                if t >= 1:
                    u = wp.tile([128, 512], BF, tag="u")
                    nc.scalar.activation(u, pu, AF.Tanh)
                    scr = wp.tile([128, 512], BF, tag="scr")
                    nc.vector.scalar_tensor_tensor(
                        out=scr, in0=u, scalar=1.0, in1=vb,
                        op0=ALU.mult, op1=ALU.mult,
                        accum_out=scores[:, t - 1:t])

                # --- keep-warm dummy matmuls: fill the end-of-iteration
                # PE stall so HAM never sees an idle window (gated on
                # h_new so they land in the stall, and emitted after the
                # u-tanh read of pu so the garbage write is safe) ---
                if t >= 1:
                    nc.tensor.matmul(pu[:, 0:512], lhsT=h_new[:, 0:128],
                                     rhs=waT[:, 0:512], start=True, stop=True,
                                     skip_group_check=True)
                    nc.tensor.matmul(pu[:, 0:512], lhsT=h_new[:, 256:384],
                                     rhs=waT[:, 512:1024], start=True, stop=True,
                                     skip_group_check=True)

                # --- batched exp of scores, every 4 completed steps ---
                # exp(s) = 1/sigmoid(-s) - 1 (stays in the sigmoid table set)
                done = t
                if done % 4 == 0 and done > 0:
                    j = done // 4 - 1
                    sl = slice(j * 4, (j + 1) * 4)
                    nc.scalar.activation(separts[:, 0:4], scores[:, sl],
                                         AF.Sigmoid, scale=-1.0)
                    nc.vector.reciprocal(separts[:, 4:8], separts[:, 0:4])
                    nc.vector.tensor_scalar_add(ew[:, sl], separts[:, 4:8], -1.0)

                # --- lagged weighted-sum into resident pwa ---
                if wsum_p < (t // 4) * 4:
                    wsum_step()
                    wsum_p += 1

                # --- DMA: prefetch gather for step t+PRE ---
                if t + PRE < 32:
                    g = wgi.tile([128, 1536], BF, tag="gi")
                    nc.gpsimd.indirect_dma_start(
                        out=g[:, :], out_offset=None, in_=G_ap[:, :],
                        in_offset=bass.IndirectOffsetOnAxis(
                            ap=toks[:, t + PRE:t + PRE + 1], axis=0),
                    )
                    gi_tiles[t + PRE] = g

            # ---- word epilogue: attention for t=31 + remaining wsum ----
            nc.tensor.matmul(pu, lhsT=ones, rhs=barow, start=True, stop=False)
            for j in range(4):
                nc.tensor.matmul(pu, lhsT=prev_hT[:, j * 128:(j + 1) * 128],
                                 rhs=waT[:, j * 512:(j + 1) * 512],
                                 start=False, stop=(j == 3))
            u = wp.tile([128, 512], BF, tag="u")
            nc.scalar.activation(u, pu, AF.Tanh)
            scr = wp.tile([128, 512], BF, tag="scr")
            nc.vector.scalar_tensor_tensor(
                out=scr, in0=u, scalar=1.0, in1=vb,
                op0=ALU.mult, op1=ALU.mult, accum_out=scores[:, 31:32])
            nc.scalar.activation(separts[:, 0:4], scores[:, 28:32],
                                 AF.Sigmoid, scale=-1.0)
            nc.vector.reciprocal(separts[:, 4:8], separts[:, 0:4])
            nc.vector.tensor_scalar_add(ew[:, 28:32], separts[:, 4:8], -1.0)
            while wsum_p < 32:
                wsum_step()
                wsum_p += 1
            # normalize: sent = pwa / sum(exp)
            se = wp.tile([128, 1], F32, tag="se")
            nc.vector.tensor_reduce(se, ew, axis=mybir.AxisListType.X,
                                    op=ALU.add)
            rse = wp.tile([128, 1], F32, tag="rse")
            nc.vector.reciprocal(rse, se)
            nc.vector.tensor_scalar_mul(sent, pwa, rse)

        # ---- sent -> sentT + sentence input projections ----
        with tc.tile_pool(name="mid", bufs=1) as mp, \
             tc.tile_pool(name="pmid", bufs=1, space="PSUM") as pmp:
            ptm = pmp.tile([128, 512], BF, tag="ptm")
            for j in range(4):
                nc.tensor.transpose(ptm[:, j * 128:(j + 1) * 128],
                                    in_=sent[:, j * 128:(j + 1) * 128],
                                    identity=ident)
            sentT = mp.tile([128, 512], BF)
            nc.vector.tensor_copy(sentT[:, 0:256], ptm[:, 0:256])
            nc.scalar.copy(sentT[:, 256:512], ptm[:, 256:512])

            psg = pmp.tile([128, 1536], F32, tag="psg")
            for ns in range(3):
                sl = slice(ns * 512, (ns + 1) * 512)
                nc.tensor.matmul(psg[:, sl], lhsT=ones, rhs=sprow[:, sl],
                                 start=True, stop=False)
                for k in range(4):
                    nc.tensor.matmul(psg[:, sl],
                                     lhsT=sentT[:, k * 128:(k + 1) * 128],
                                     rhs=swihT[:, k * 1536 + ns * 512:
                                               k * 1536 + (ns + 1) * 512],
                                     start=False, stop=(k == 3))
            nc.scalar.copy(sgi[:, 0:512], psg[:, 0:512])
            nc.scalar.copy(sgi[:, 512:1024], psg[:, 512:1024])
            nc.vector.tensor_copy(sgi[:, 1024:1536], psg[:, 1024:1536])
            # transpose sgi into feature-major gate-chunk slots, grouped
            # per direction:
            # [d0: r0 r1 z0 z1 | d1: r0 r1 z0 z1 | d0: n0 n1 | d1: n0 n1]
            slot_src = [0, 128, 256, 384, 512, 640, 768, 896,
                        1024, 1152, 1280, 1408]
            ptg = pmp.tile([128, 1536], BF, tag="ptg")
            for j, src in enumerate(slot_src):
                nc.tensor.transpose(ptg[:, j * 128:(j + 1) * 128],
                                    in_=sgi[:, src:src + 128],
                                    identity=ident)
            psgT3c = psgT.rearrange("p (g r) -> p g r", g=12)
            ptg3c = ptg.rearrange("p (g r) -> p g r", g=12)
            nc.scalar.copy(psgT3c[:, :, 0:64], ptg3c[:, :, 0:64])
            nc.vector.tensor_copy(psgT3c[:, :, 64:128], ptg3c[:, :, 64:128])

        # ================= sentence stage =================
        # Feature-major sentence GRU: state h^T [128 feat(chunk d,m), 8 docs],
        # weight-stationary rec matmuls (N=8), gate math at FD<=32,
        # directions staggered so d1's matmuls stream during d0's gate math.
        with tc.tile_pool(name="sp", bufs=3) as sp:
          with tc.tile_pool(name="pgs", bufs=2, space="PSUM") as pgsp:
            psgT3 = psgT.rearrange("p (g r) -> p g r", g=12)

            for t in range(16):
                hprev = hsf_hist[:, t * 32:(t + 1) * 32]
                h_new = hsf_hist[:, (t + 1) * 32:(t + 2) * 32]
                prz_full = pgsp.tile([128, 512], F32, tag="prz")
                pnn_full = pgsp.tile([128, 512], F32, tag="pnn")
                prz = prz_full[:, 0:64]   # [d0: r0 r1 z0 z1 | d1: ...] x 8
                pnn = pnn_full[:, 0:32]   # [d0: n0 n1 | d1: n0 n1] x 8

                for d in range(2):
                    # --- PE: bias opener + recurrent matmuls, dir d ---
                    nc.tensor.matmul(pnn[:, d * 16:(d + 1) * 16], lhsT=sbrowF,
                                     rhs=bones[:, d * 16:(d + 1) * 16],
                                     start=True, stop=False)
                    for sl_ in range(4):  # (g in r,z) x (m in 0,1)
                        for k in range(2):
                            w_i = d * 8 + sl_ * 2 + k
                            w = swhhF[:, w_i * 128:(w_i + 1) * 128]
                            nc.tensor.matmul(
                                prz[:, d * 32 + sl_ * 8:d * 32 + (sl_ + 1) * 8],
                                lhsT=w,
                                rhs=hprev[:, (d * 2 + k) * 8:(d * 2 + k + 1) * 8],
                                start=(k == 0), stop=(k == 1))
                    for m in range(2):
                        for k in range(2):
                            w_i = 16 + d * 4 + m * 2 + k
                            w = swhhF[:, w_i * 128:(w_i + 1) * 128]
                            nc.tensor.matmul(
                                pnn[:, d * 16 + m * 8:d * 16 + (m + 1) * 8],
                                lhsT=w,
                                rhs=hprev[:, (d * 2 + k) * 8:(d * 2 + k + 1) * 8],
                                start=False, stop=(m == 1 and k == 1))

                    # --- gate math, dir d (tiny FD; runs while other dir's
                    #     matmuls stream on PE) ---
                    przd = prz[:, d * 32:(d + 1) * 32]
                    pnnd = pnn[:, d * 16:(d + 1) * 16]
                    hprevd = hprev[:, d * 16:(d + 1) * 16]
                    hnewd = h_new[:, d * 16:(d + 1) * 16]
                    rzp = sp.tile([128, 32], BF, tag=f"srzp{d}")
                    nc.vector.tensor_tensor(
                        rzp.rearrange("p (g r) -> p g r", g=4),
                        przd.rearrange("p (g r) -> p g r", g=4),
                        psgT3[:, d * 4:(d + 1) * 4, t * 8:(t + 1) * 8],
                        op=ALU.add)
                    rzd = sp.tile([128, 32], BF, tag=f"srz{d}")
                    nc.scalar.activation(rzd, rzp, AF.Sigmoid)
                    t1 = sp.tile([128, 16], BF, tag=f"st1{d}")
                    nc.vector.tensor_tensor(t1, rzd[:, 0:16], pnnd, op=ALU.mult)
                    npre = sp.tile([128, 16], BF, tag=f"snp{d}")
                    nc.vector.tensor_tensor(
                        npre.rearrange("p (g r) -> p g r", g=2),
                        t1.rearrange("p (g r) -> p g r", g=2),
                        psgT3[:, 8 + d * 2:8 + (d + 1) * 2, t * 8:(t + 1) * 8],
                        op=ALU.add)
                    omz = sp.tile([128, 16], BF, tag=f"som{d}")
                    nc.gpsimd.tensor_scalar(out=omz, in0=rzd[:, 16:32],
                                            scalar1=-1.0, scalar2=1.0,
                                            op0=ALU.mult, op1=ALU.add)
                    zh = sp.tile([128, 16], BF, tag=f"szh{d}")
                    nc.gpsimd.tensor_tensor(zh, rzd[:, 16:32], hprevd,
                                            op=ALU.mult)
                    nnT = sp.tile([128, 16], BF, tag=f"snn{d}")
                    nc.scalar.activation(nnT, npre, AF.Tanh)
                    nom = sp.tile([128, 16], BF, tag=f"snm{d}")
                    nc.vector.tensor_tensor(nom, nnT, omz, op=ALU.mult)
                    nc.vector.tensor_add(hnewd, nom, zh)

                    # batch layout for the attention post-pass
                    hsbT4 = hsbT.rearrange("p (d c r) -> p d c r", d=2, c=2)
                    nc.scalar.copy(
                        hsbT4[:, d, :, 8 * t:8 * (t + 1)],
                        hnewd.rearrange("p (c r) -> p c r", c=2))

            # rebuild batch-major hsb from hsbT (4 transposes)
            pth = pgsp.tile([128, 512], BF, tag="pth")
            for j in range(4):
                nc.tensor.transpose(pth[:, j * 128:(j + 1) * 128],
                                    in_=hsbT[:, j * 128:(j + 1) * 128],
                                    identity=ident)
            nc.vector.tensor_copy(hsb[:, 0:256], pth[:, 0:256])
            nc.scalar.copy(hsb[:, 256:512], pth[:, 256:512])

          # ---- sentence attention post-pass (batch rows = (s,d)) ----
          with tc.tile_pool(name="pps", bufs=1, space="PSUM") as ppsp:
            pu2 = ppsp.tile([128, 512], F32, tag="pu2")
            nc.tensor.matmul(pu2, lhsT=ones, rhs=sbarow, start=True, stop=False)
            for j in range(4):
                nc.tensor.matmul(pu2, lhsT=hsbT[:, j * 128:(j + 1) * 128],
                                 rhs=sawT[:, j * 512:(j + 1) * 512],
                                 start=False, stop=(j == 3))
            u2 = sp.tile([128, 512], BF, tag="u2")
            nc.scalar.activation(u2, pu2, AF.Tanh)
            s_sc = sp.tile([128, 1], F32, tag="s_sc")
            scr2 = sp.tile([128, 512], BF, tag="scr2")
            nc.vector.scalar_tensor_tensor(
                out=scr2, in0=u2, scalar=1.0, in1=svb,
                op0=ALU.mult, op1=ALU.mult, accum_out=s_sc)
            # exp via sigmoid set: exp(s) = 1/sigmoid(-s) - 1
            ew2s = sp.tile([128, 1], F32, tag="ew2s")
            nc.scalar.activation(ew2s, s_sc, AF.Sigmoid, scale=-1.0)
            ew2r = sp.tile([128, 1], F32, tag="ew2r")
            nc.vector.reciprocal(ew2r, ew2s)
            ew2 = sp.tile([128, 1], F32, tag="ew2")
            nc.vector.tensor_scalar_add(ew2, ew2r, -1.0)
            # per-doc sums via indicator matmul, then broadcast back
            pds = ppsp.tile([8, 1], F32, tag="pds")
            nc.tensor.matmul(pds, lhsT=ind8f, rhs=ew2, start=True, stop=True)
            dsum = sp.tile([8, 1], F32, tag="dsum")
            nc.vector.tensor_copy(dsum, pds)
            rds = sp.tile([8, 1], F32, tag="rds")
            nc.vector.reciprocal(rds, dsum)
            prb = ppsp.tile([128, 1], F32, tag="prb")
            nc.tensor.matmul(prb, lhsT=ind8T, rhs=rds, start=True, stop=True)
            aw = sp.tile([128, 1], F32, tag="aw")
            nc.vector.tensor_tensor(aw, ew2, prb, op=ALU.mult)
            awd = sp.tile([128, 8], BF, tag="awd")
            nc.vector.tensor_scalar_mul(awd, ind8, aw)
            pdoc = ppsp.tile([8, 512], F32, tag="pdoc")
            nc.tensor.matmul(pdoc, lhsT=awd, rhs=hsb, start=True, stop=True)
            doc = sp.tile([8, 512], BF, tag="doc")
            nc.scalar.copy(doc, pdoc)

            # ---- classifier + log_softmax ----
            ptd = ppsp.tile([128, 32], BF, tag="ptd")
            for j in range(4):
                nc.tensor.transpose(ptd[:, j * 8:(j + 1) * 8],
                                    in_=doc[:, j * 128:(j + 1) * 128],
                                    identity=ident[0:8, 0:8])
            docT = sp.tile([128, 32], BF, tag="docT")
            nc.vector.tensor_copy(docT, ptd)
            pl = ppsp.tile([8, NCLS], F32, tag="pl")
            nc.tensor.matmul(pl, lhsT=ones[:, 0:8], rhs=fcb,
                             start=True, stop=False)
            for j in range(4):
                nc.tensor.matmul(pl, lhsT=docT[:, j * 8:(j + 1) * 8],
                                 rhs=fcwT[:, j * NCLS:(j + 1) * NCLS],
                                 start=False, stop=(j == 3))
            nmx2 = sp.tile([8, 1], F32, tag="nmx2")
            nc.vector.tensor_reduce(nmx2, pl, axis=mybir.AxisListType.X,
                                    op=ALU.max, negate=True)
            e2 = sp.tile([8, NCLS], F32, tag="e2")
            se2 = sp.tile([8, 1], F32, tag="se2")
            nc.scalar.activation(e2, pl, AF.Exp, bias=nmx2, accum_out=se2)
            lse = sp.tile([8, 1], F32, tag="lse")
            nc.scalar.activation(lse, se2, AF.Ln)
            out_sb = sp.tile([8, NCLS], F32, tag="out_sb")
            nc.vector.tensor_scalar(out=out_sb, in0=pl, scalar1=nmx2,
                                    scalar2=lse, op0=ALU.add, op1=ALU.subtract)
            nc.sync.dma_start(out=dram("out"), in_=out_sb)


# ---------------------------------------------------------------------------
# host side
# ---------------------------------------------------------------------------

def _prep_inputs(inputs):
    """Build the per-core in_maps (host preprocessing + sharding)."""
    f32 = np.float32
    emb = np.asarray(inputs["emb"], f32)
    w_Wih = np.asarray(inputs["w_Wih"], f32)
    w_Whh = np.asarray(inputs["w_Whh"], f32)
    w_bih = np.asarray(inputs["w_bih"], f32)
    w_bhh = np.asarray(inputs["w_bhh"], f32)
    wa_W = np.asarray(inputs["wa_W"], f32)
    wa_b = np.asarray(inputs["wa_b"], f32)
    wa_v = np.asarray(inputs["wa_v"], f32)
    s_Wih = np.asarray(inputs["s_Wih"], f32)
    s_Whh = np.asarray(inputs["s_Whh"], f32)
    s_bih = np.asarray(inputs["s_bih"], f32)
    s_bhh = np.asarray(inputs["s_bhh"], f32)
    sa_W = np.asarray(inputs["sa_W"], f32)
    sa_b = np.asarray(inputs["sa_b"], f32)
    sa_v = np.asarray(inputs["sa_v"], f32)
    fc_W = np.asarray(inputs["fc_W"], f32)
    fc_b = np.asarray(inputs["fc_b"], f32)
    tokens = np.asarray(inputs["tokens"])

    def b(x):
        return np.ascontiguousarray(x.astype(bf16))

    # folded gather table G [V, 1536] = [rz0 | rz1 | n0 | n1]
    g0 = emb @ w_Wih[0].T + w_bih[0]
    g0[:, :512] += w_bhh[0][:512]
    g1 = emb @ w_Wih[1].T + w_bih[1]
    g1[:, :512] += w_bhh[1][:512]
    G = np.concatenate([g0[:, :512], g1[:, :512], g0[:, 512:], g1[:, 512:]], 1)

    whhT = np.stack([w_Whh[0].T[:128], w_Whh[0].T[128:],
                     w_Whh[1].T[:128], w_Whh[1].T[128:]])  # [4,128,768]
    brow = np.concatenate([w_bhh[0][512:], w_bhh[1][512:]])[None, :]
    vbc = np.broadcast_to(wa_v, (128, 512))

    # sentence input-proj table [512, 1536] with same col layout; bias row
    sg0 = s_Wih[0].T  # [512, 768]
    sg1 = s_Wih[1].T
    swihT = np.concatenate([sg0[:, :512], sg1[:, :512],
                            sg0[:, 512:], sg1[:, 512:]], 1)
    sprow = np.concatenate([
        s_bih[0][:512] + s_bhh[0][:512],
        s_bih[1][:512] + s_bhh[1][:512],
        s_bih[0][512:], s_bih[1][512:]])[None, :]
    # feature-major weight chunks for the sentence GRU, slot order grouped
    # by direction: rz: for d: for g in (r,z): for m: for k;
    # n: for d: for m: for k
    goff = {"r": 0, "z": 256, "n": 512}
    fch = []
    for d in range(2):
        for g in ("r", "z"):
            for m in range(2):
                for k in range(2):
                    fch.append(s_Whh[d][goff[g] + m * 128:goff[g] + (m + 1) * 128,
                                        k * 128:(k + 1) * 128].T)
    for d in range(2):
        for m in range(2):
            for k in range(2):
                fch.append(s_Whh[d][goff["n"] + m * 128:goff["n"] + (m + 1) * 128,
                                    k * 128:(k + 1) * 128].T)
    swhhF = np.stack(fch)  # [24, 128, 128]
    sbrow = np.concatenate([s_bhh[0][512:], s_bhh[1][512:]])[None, :]
    svbc = np.broadcast_to(sa_v, (128, 512))

    ind = np.zeros((128, 8), f32)
    for row in range(128):
        ind[row, row % 8] = 1.0

    shared = {
        "G": b(G), "whhT": b(whhT), "brow": b(brow),
        "waT": b(wa_W.T), "barow": b(wa_b[None, :]), "vb": b(vbc),
        "swihT": b(swihT), "sprow": b(sprow),
        "swhhF": b(swhhF),
        "sbrowF": b(sbrow.reshape(4, 128)),
        "bones": b(np.repeat(np.eye(4, dtype=f32), 8, axis=1)),
        "sbrow": b(sbrow), "sawT": b(sa_W.T), "sbarow": b(sa_b[None, :]),
        "svb": b(svbc), "fcwT": b(fc_W.T), "fcb": b(fc_b[None, :]),
        "ind8": b(ind), "ind8f": np.ascontiguousarray(ind),
        "ind8T": np.ascontiguousarray(ind.T),
    }
    in_maps = []
    for c in range(NCORES):
        # word-row p = s*8 + doc  (so sentence step s owns partition rows
        # [s*8:(s+1)*8] of the batch-major sentence matrix)
        tk = np.ascontiguousarray(
            np.transpose(tokens[c * BC:(c + 1) * BC], (1, 0, 2))
            .reshape(NW, W).astype(np.int32))
        in_maps.append({**shared, "toks": tk})
    return in_maps


_NC_CACHE = {}


def _get_nc():
    if "nc" not in _NC_CACHE:
        _NC_CACHE["nc"] = _build_program()
    return _NC_CACHE["nc"]


def kernel(**inputs) -> np.ndarray:
    nc = _get_nc()
    in_maps = _prep_inputs(inputs)
    res = bass_utils.run_bass_kernel_spmd(nc, in_maps, core_ids=list(range(NCORES)))
    outs = []
    for c in range(NCORES):
        o = np.asarray(res.results[c]["out"], np.float32)
        outs.append(o)
    return np.concatenate(outs, 0)


# revision 24
# speedup vs baseline: 1.3439x; 1.0219x over previous
"""HAN (hierarchical attention network) forward pass on 8 TRN2 NeuronCores.

Strategy
--------
Data-parallel over batch: each core handles 8 documents = 128 sentences =
4096 tokens, fully independently (no collectives). Inside a core:

* Embedding lookup + word-GRU input projection folded on host:
  gi = (emb @ Wih.T)[tokens], gathered per step with indirect DMA.
* Word bi-GRU, batch-major [128 sentences, feat]. Per-direction gate math
  so the two independent direction recurrences pipeline across ACT/DVE/
  GPSIMD. Engine programs are ordered so the d0 chain (rec matmul ->
  sigmoid -> r*hn -> +gi -> tanh -> blend -> transpose -> copy) never
  waits behind low-priority work; attention and injects fill PE gaps.
* gi injection PSUM groups are single-buffered (inject for t+1 reuses the
  banks right after sigmoid(t) reads them), freeing banks for TWO
  transpose banks (pt0/pt1) so hT copies overlap the second transpose
  pair, and for a RESIDENT weighted-sum accumulator bank (diag(exp(s))
  matmuls accumulate across all 32 steps; no per-4-step PSUM drain).
* Word attention: scores accumulated in-loop (u = tanh(h Wa + ba) lagged
  one step; u.v via stt-accum). Softmax without max-subtraction as an
  exp-weighted running matmul sum.
* Sentence bi-GRU: feature-major (free dims 16-32), the two directions
  emitted staggered: d1's 12 weight-load+matmul pairs stream on PE while
  d0's gate math runs on ACT/DVE, and vice versa. Sentence attention is
  a post-pass batch GEMM; per-document softmax via indicator matmuls.

Compute dtype bf16 (fp32 PSUM accumulation + fp32 attention
accumulators).
"""

import numpy as np
import ml_dtypes

import concourse.bass as bass
import concourse.mybir as mybir
import concourse.tile as tile
from concourse import bacc, bass_utils
from concourse.masks import make_identity

BF = mybir.dt.bfloat16
F32 = mybir.dt.float32
AF = mybir.ActivationFunctionType
ALU = mybir.AluOpType
bf16 = ml_dtypes.bfloat16

V, E = 50000, 300
HW_, HS_ = 256, 256
NCLS = 10
B, S, W = 64, 16, 32
NCORES = 8
BC = B // NCORES          # docs per core = 8
NW = BC * S               # word-level batch per core = 128
GW = 3 * HW_              # 768


def _build_program():
    nc = bacc.Bacc(
        "TRN2",
        target_bir_lowering=False,
        debug=False,
        enable_asserts=False,
        num_devices=NCORES,
    )

    # ---- DRAM I/O ----
    h = {}
    h["G"] = nc.dram_tensor("G", [V, 1536], BF, kind="ExternalInput")
    h["toks"] = nc.dram_tensor("toks", [128, 32], mybir.dt.int32, kind="ExternalInput")
    h["whhT"] = nc.dram_tensor("whhT", [4, 128, GW], BF, kind="ExternalInput")
    h["brow"] = nc.dram_tensor("brow", [1, 512], BF, kind="ExternalInput")
    h["waT"] = nc.dram_tensor("waT", [512, 512], BF, kind="ExternalInput")
    h["barow"] = nc.dram_tensor("barow", [1, 512], BF, kind="ExternalInput")
    h["vb"] = nc.dram_tensor("vb", [128, 512], BF, kind="ExternalInput")
    h["swihT"] = nc.dram_tensor("swihT", [512, 1536], BF, kind="ExternalInput")
    h["sprow"] = nc.dram_tensor("sprow", [1, 1536], BF, kind="ExternalInput")
    h["swhhF"] = nc.dram_tensor("swhhF", [24, 128, 128], BF, kind="ExternalInput")
    h["sbrowF"] = nc.dram_tensor("sbrowF", [4, 128], BF, kind="ExternalInput")
    h["bones"] = nc.dram_tensor("bones", [4, 32], BF, kind="ExternalInput")
    h["sbrow"] = nc.dram_tensor("sbrow", [1, 512], BF, kind="ExternalInput")
    h["sawT"] = nc.dram_tensor("sawT", [512, 512], BF, kind="ExternalInput")
    h["sbarow"] = nc.dram_tensor("sbarow", [1, 512], BF, kind="ExternalInput")
    h["svb"] = nc.dram_tensor("svb", [128, 512], BF, kind="ExternalInput")
    h["ind8"] = nc.dram_tensor("ind8", [128, 8], BF, kind="ExternalInput")
    h["ind8f"] = nc.dram_tensor("ind8f", [128, 8], F32, kind="ExternalInput")
    h["ind8T"] = nc.dram_tensor("ind8T", [8, 128], F32, kind="ExternalInput")
    h["fcwT"] = nc.dram_tensor("fcwT", [512, NCLS], BF, kind="ExternalInput")
    h["fcb"] = nc.dram_tensor("fcb", [1, NCLS], BF, kind="ExternalInput")
    h["out"] = nc.dram_tensor("out", [BC, NCLS], F32, kind="ExternalOutput")

    with tile.TileContext(nc) as tc:
        _body(nc, tc, h)
    nc.compile()
    return nc


def _body(nc, tc, handles):
    def dram(name):
        return handles[name].ap()

    G_ap = dram("G")
    with tc.tile_pool(name="const", bufs=1) as cp:
        # ---- constants / weights in SBUF ----
        ident = cp.tile([128, 128], BF)
        make_identity(nc, ident)
        ones = cp.tile([1, 128], BF)
        nc.gpsimd.memset(ones, 1.0)

        toks = cp.tile([128, 32], mybir.dt.int32)
        nc.sync.dma_start(out=toks, in_=dram("toks"))
        # initial gathers issued before the weight-DMA flood so their
        # completion semaphores only cover the toks DMA
        gi_init = []
        for t0_ in range(3):
            g0_ = cp.tile([128, 1536], BF, name=f"gi_init{t0_}")
            nc.gpsimd.indirect_dma_start(
                out=g0_[:, :], out_offset=None, in_=G_ap[:, :],
                in_offset=bass.IndirectOffsetOnAxis(ap=toks[:, t0_:t0_ + 1],
                                                    axis=0),
            )
            gi_init.append(g0_)
        whh = cp.tile([128, 4 * GW], BF)  # (d0k0 d0k1 d1k0 d1k1); [rz(512) n(256)]
        for j in range(4):
            nc.sync.dma_start(out=whh[:, j * GW:(j + 1) * GW],
                              in_=dram("whhT")[j])
        brow = cp.tile([1, 512], BF)
        nc.sync.dma_start(out=brow, in_=dram("brow"))
        waT = cp.tile([128, 4 * 512], BF)
        for j in range(4):
            nc.sync.dma_start(out=waT[:, j * 512:(j + 1) * 512],
                              in_=dram("waT")[j * 128:(j + 1) * 128, :])
        barow = cp.tile([1, 512], BF)
        nc.sync.dma_start(out=barow, in_=dram("barow"))
        vb = cp.tile([128, 512], BF)
        nc.sync.dma_start(out=vb, in_=dram("vb"))

        swihT = cp.tile([128, 4 * 1536], BF)
        for j in range(4):
            nc.sync.dma_start(out=swihT[:, j * 1536:(j + 1) * 1536],
                              in_=dram("swihT")[j * 128:(j + 1) * 128, :])
        sprow = cp.tile([1, 1536], BF)
        nc.sync.dma_start(out=sprow, in_=dram("sprow"))
        swhhF = cp.tile([128, 24 * 128], BF)
        for j in range(24):
            nc.sync.dma_start(out=swhhF[:, j * 128:(j + 1) * 128],
                              in_=dram("swhhF")[j])
        sbrow = cp.tile([1, 512], BF)
        nc.sync.dma_start(out=sbrow, in_=dram("sbrow"))
        sbrowF = cp.tile([4, 128], BF)
        nc.sync.dma_start(out=sbrowF, in_=dram("sbrowF"))
        bones = cp.tile([4, 32], BF)
        nc.sync.dma_start(out=bones, in_=dram("bones"))
        sawT = cp.tile([128, 4 * 512], BF)
        for j in range(4):
            nc.sync.dma_start(out=sawT[:, j * 512:(j + 1) * 512],
                              in_=dram("sawT")[j * 128:(j + 1) * 128, :])
        sbarow = cp.tile([1, 512], BF)
        nc.sync.dma_start(out=sbarow, in_=dram("sbarow"))
        svb = cp.tile([128, 512], BF)
        nc.sync.dma_start(out=svb, in_=dram("svb"))
        ind8 = cp.tile([128, 8], BF)
        nc.sync.dma_start(out=ind8, in_=dram("ind8"))
        ind8f = cp.tile([128, 8], F32)
        nc.sync.dma_start(out=ind8f, in_=dram("ind8f"))
        ind8T = cp.tile([8, 128], F32)
        nc.sync.dma_start(out=ind8T, in_=dram("ind8T"))
        fcwT = cp.tile([128, 4 * NCLS], BF)
        for j in range(4):
            nc.sync.dma_start(out=fcwT[:, j * NCLS:(j + 1) * NCLS],
                              in_=dram("fcwT")[j * 128:(j + 1) * 128, :])
        fcb = cp.tile([1, NCLS], BF)
        nc.sync.dma_start(out=fcb, in_=dram("fcb"))

        # ---- persistent state ----
        hw_hist = cp.tile([128, 33 * 512], BF)   # h_t history, slot 0 = zeros
        nc.gpsimd.memset(hw_hist[:, 0:512], 0.0)
        hT0 = cp.tile([128, 512], BF)            # transposed h state, step -1
        nc.gpsimd.memset(hT0, 0.0)
        scores = cp.tile([128, 32], F32)
        ew = cp.tile([128, 32], F32)             # exp(scores)
        separts = cp.tile([128, 8], F32)         # partial exp sums (per 4-batch)
        # tiny dummy sigmoid: pulls the ACT_TABLE_LOAD for the sigmoid set
        # to kernel start, overlapping it with the weight DMAs
        nc.scalar.activation(separts[:, 0:1], ident[:, 0:1], AF.Sigmoid)
        sent = cp.tile([128, 512], BF)           # word-attention output
        sgi = cp.tile([128, 1536], BF)           # sentence-GRU input projections
        psgT = cp.tile([128, 12 * 128], BF)      # transposed gi: gate-chunk x rows
        hsf_hist = cp.tile([128, 17 * 32], BF)   # feature-major h^T history
        nc.gpsimd.memset(hsf_hist[:, 0:32], 0.0)
        hsb = cp.tile([128, 512], BF)            # sentence h, batch rows (s,d)
        hsbT = cp.tile([128, 4 * 128], BF)       # transposed: feat-chunk x rows

        # ================= word stage =================
        with tc.tile_pool(name="wp", bufs=3) as wp, \
             tc.tile_pool(name="wgi", bufs=5) as wgi, \
             tc.tile_pool(name="pp", bufs=1, space="PSUM") as pp:

            # PSUM banks (all resident for the whole loop):
            pga = pp.tile([128, 512], F32)   # rz pre-acts dir0
            pgb = pp.tile([128, 512], F32)   # rz pre-acts dir1
            pn = pp.tile([128, 512], F32)    # n pre-acts (both dirs)
            pu0 = pp.tile([128, 512], F32)   # attention u pre-acts (ping)
            pu1 = pp.tile([128, 512], F32)   # attention u pre-acts (pong)
            pwa = pp.tile([128, 512], F32)   # resident exp-weighted h sum
            # transposes: two tiles padded to full banks so DVE reads of
            # pt0 can overlap PE writes of pt1 (no same-bank collision)
            pt0 = pp.tile([128, 256], BF, padded_shape=[128, 1024])
            pt1 = pp.tile([128, 256], BF, padded_shape=[128, 1024])

            PRE = 3  # gather prefetch depth
            gi_tiles = {t: gi_init[t] for t in range(PRE)}

            def inject(t):
                """Open PSUM accumulation groups for step t with gi + biases."""
                gi = gi_tiles[t]
                nc.tensor.matmul(pga, lhsT=ident, rhs=gi[:, 0:512],
                                 start=True, stop=False)
                nc.tensor.matmul(pgb, lhsT=ident, rhs=gi[:, 512:1024],
                                 start=True, stop=False)
                nc.tensor.matmul(pn, lhsT=ones, rhs=brow,
                                 start=True, stop=False)

            inject(0)

            prev_hT = hT0
            wsum_p = 0  # next pending weighted-sum step

            def wsum_step():
                """Accumulate one lagged exp-weighted h into resident pwa."""
                s = wsum_p
                dg = wp.tile([128, 128], BF, tag="dg")
                nc.vector.tensor_scalar_mul(dg, ident, ew[:, s:s + 1])
                nc.tensor.matmul(pwa, lhsT=dg,
                                 rhs=hw_hist[:, (s + 1) * 512:(s + 2) * 512],
                                 start=(s == 0), stop=(s == 31),
                                 skip_group_check=True)

            def attn(s):
                """u pre-acts for step s into pu[s % 2] (ping-pong)."""
                pu = pu0 if s % 2 == 0 else pu1
                nc.tensor.matmul(pu, lhsT=ones, rhs=barow,
                                 start=True, stop=False)
                for j in range(4):
                    nc.tensor.matmul(pu, lhsT=prev_hT[:, j * 128:(j + 1) * 128],
                                     rhs=waT[:, j * 512:(j + 1) * 512],
                                     start=False, stop=(j == 3))

            def utanh_stt(s):
                """u(s) = tanh(pu[s%2]); score[s] via stt-accum. The tanh is
                lag-2 so it slots into the sigmoid->tanh ACT gap instead of
                delaying the gate chain."""
                pu = pu0 if s % 2 == 0 else pu1
                u = wp.tile([128, 512], BF, tag="u")
                nc.scalar.activation(u, pu, AF.Tanh)
                return u

            def score_stt(s, u):
                scr = wp.tile([128, 512], BF, tag="scr")
                nc.vector.scalar_tensor_tensor(
                    out=scr, in0=u, scalar=1.0, in1=vb,
                    op0=ALU.mult, op1=ALU.mult,
                    accum_out=scores[:, s:s + 1])

            for t in range(32):
                gi = gi_tiles.pop(t)
                ds = (0, 1)
                pg = {0: pga, 1: pgb}

                # --- PE: recurrent matmuls for step t (need prev_hT) ---
                for dd in ds:
                    for k in range(2):
                        lhs = prev_hT[:, (dd * 2 + k) * 128:(dd * 2 + k + 1) * 128]
                        w = whh[:, (dd * 2 + k) * GW:(dd * 2 + k + 1) * GW]
                        nc.tensor.matmul(pg[dd], lhsT=lhs, rhs=w[:, 0:512],
                                         start=False, stop=(k == 1))
                for dd in ds:
                    for k in range(2):
                        lhs = prev_hT[:, (dd * 2 + k) * 128:(dd * 2 + k + 1) * 128]
                        w = whh[:, (dd * 2 + k) * GW:(dd * 2 + k + 1) * GW]
                        nc.tensor.matmul(pn[:, dd * 256:(dd + 1) * 256],
                                         lhsT=lhs, rhs=w[:, 512:768],
                                         start=False,
                                         stop=(dd == ds[1] and k == 1))

                # --- ACT: sigmoids (start of per-dir gate chains), then
                #     the lag-2 attention tanh in the sigmoid->tanh gap ---
                rz = wp.tile([128, 1024], BF, tag="rz")  # [r0 z0 | r1 z1]
                for dd in ds:
                    nc.scalar.activation(rz[:, dd * 512:(dd + 1) * 512],
                                         pg[dd], AF.Sigmoid)
                u_lag = utanh_stt(t - 2) if t >= 2 else None

                h_prev = hw_hist[:, t * 512:(t + 1) * 512]
                h_new = hw_hist[:, (t + 1) * 512:(t + 2) * 512]
                hT = wp.tile([128, 512], BF, tag="hT")
                nn = wp.tile([128, 512], BF, tag="nn")   # [n0 n1]
                t1_ = {0: wp.tile([128, 256], BF, tag="t1a", name="t1a"),
                       1: wp.tile([128, 256], BF, tag="t1b", name="t1b")}
                np_ = {0: wp.tile([128, 256], BF, tag="npa", name="npa"),
                       1: wp.tile([128, 256], BF, tag="npb", name="npb")}
                omz = wp.tile([128, 512], BF, tag="omz")
                zh = wp.tile([128, 512], BF, tag="zh")
                nom = {0: wp.tile([128, 256], BF, tag="noma", name="noma"),
                       1: wp.tile([128, 256], BF, tag="nomb", name="nomb")}

                def rslice(dd):
                    return rz[:, dd * 512:dd * 512 + 256]

                def zslice(dd):
                    return rz[:, dd * 512 + 256:(dd + 1) * 512]

                # --- DVE gate chains (t1/np read pn; must precede the
                #     t+1 inject in program order) ---
                for dd in ds:
                    nc.vector.tensor_tensor(t1_[dd], rslice(dd),
                                            pn[:, dd * 256:(dd + 1) * 256],
                                            op=ALU.mult)
                    nc.vector.tensor_add(np_[dd], t1_[dd],
                                         gi[:, 1024 + dd * 256:
                                            1024 + (dd + 1) * 256])

                # gp: zh first (feeds the last chain op), then omz
                for dd in ds:
                    nc.gpsimd.tensor_tensor(zh[:, dd * 256:(dd + 1) * 256],
                                            zslice(dd),
                                            h_prev[:, dd * 256:(dd + 1) * 256],
                                            op=ALU.mult)
                for dd in ds:
                    nc.gpsimd.tensor_scalar(out=omz[:, dd * 256:(dd + 1) * 256],
                                            in0=zslice(dd), scalar1=-1.0,
                                            scalar2=1.0, op0=ALU.mult,
                                            op1=ALU.add)

                # --- PE: inject step t+1 (after sigmoid + t1 reads) ---
                if t < 31:
                    inject(t + 1)

                for dd in ds:
                    nc.scalar.activation(nn[:, dd * 256:(dd + 1) * 256],
                                         np_[dd], AF.Tanh)

                # --- per-dir tails + transposes (transposes outrank the
                #     attention fill on the PE queue) ---
                for i, dd in enumerate(ds):
                    nc.vector.tensor_tensor(nom[dd], nn[:, dd * 256:(dd + 1) * 256],
                                            omz[:, dd * 256:(dd + 1) * 256],
                                            op=ALU.mult)
                    nc.vector.tensor_add(h_new[:, dd * 256:(dd + 1) * 256],
                                         nom[dd], zh[:, dd * 256:(dd + 1) * 256])
                    ptd = pt0 if dd == 0 else pt1
                    nc.tensor.transpose(ptd[:, 0:128],
                                        in_=h_new[:, dd * 256:dd * 256 + 128],
                                        identity=ident)
                    nc.tensor.transpose(ptd[:, 128:256],
                                        in_=h_new[:, dd * 256 + 128:(dd + 1) * 256],
                                        identity=ident)

                for i, dd in enumerate(ds):
                    ptd = pt0 if dd == 0 else pt1
                    if i == 0:
                        nc.vector.tensor_copy(hT[:, dd * 256:(dd + 1) * 256], ptd)
                    else:
                        nc.scalar.copy(hT[:, dd * 256:(dd + 1) * 256], ptd)

                # --- PE: attention matmuls for step t-1 (low priority;
                #     fills PE gaps; uses hT(t-1) = current prev_hT) ---
                if t >= 1:
                    attn(t - 1)
                prev_hT = hT

                # --- score for step t-2 from the lag-2 u ---
                if u_lag is not None:
                    score_stt(t - 2, u_lag)

                # --- batched exp of scores, every 4 completed steps ---
                # exp(s) = 1/sigmoid(-s) - 1 (stays in the sigmoid table set)
                done = t - 1
                if done % 4 == 0 and done > 0:
                    j = done // 4 - 1
                    sl = slice(j * 4, (j + 1) * 4)
                    nc.scalar.activation(separts[:, 0:4], scores[:, sl],
                                         AF.Sigmoid, scale=-1.0)
                    nc.vector.reciprocal(separts[:, 4:8], separts[:, 0:4])
                    nc.vector.tensor_scalar_add(ew[:, sl], separts[:, 4:8], -1.0)

                # --- lagged weighted-sum into resident pwa ---
                if wsum_p < ((t - 1) // 4) * 4:
                    wsum_step()
                    wsum_p += 1

                # --- keep-warm dummy matmuls (lowest priority PE fillers;
                # they write the pu bank already consumed by the lag-2 tanh,
                # which the next attn() re-opens with start=True) ---
                pu_cur = pu0 if t % 2 == 0 else pu1
                nc.tensor.matmul(pu_cur[:, 0:512], lhsT=ident,
                                 rhs=waT[:, 0:512], start=True, stop=True,
                                 skip_group_check=True)
                nc.tensor.matmul(pu_cur[:, 0:512], lhsT=ident,
                                 rhs=waT[:, 512:1024], start=True, stop=True,
                                 skip_group_check=True)

                # --- DMA: prefetch gather for step t+PRE ---
                if t + PRE < 32:
                    g = wgi.tile([128, 1536], BF, tag="gi")
                    nc.gpsimd.indirect_dma_start(
                        out=g[:, :], out_offset=None, in_=G_ap[:, :],
                        in_offset=bass.IndirectOffsetOnAxis(
                            ap=toks[:, t + PRE:t + PRE + 1], axis=0),
                    )
                    gi_tiles[t + PRE] = g

            # ---- word epilogue: lag-2 scores for t=30,31 + attn(31) ----
            u30 = utanh_stt(30)
            score_stt(30, u30)
            attn(31)
            u31 = utanh_stt(31)
            score_stt(31, u31)
            nc.scalar.activation(separts[:, 0:4], scores[:, 28:32],
                                 AF.Sigmoid, scale=-1.0)
            nc.vector.reciprocal(separts[:, 4:8], separts[:, 0:4])
            nc.vector.tensor_scalar_add(ew[:, 28:32], separts[:, 4:8], -1.0)
            while wsum_p < 32:
                wsum_step()
                wsum_p += 1
            # normalize: sent = pwa / sum(exp)
            se = wp.tile([128, 1], F32, tag="se")
            nc.vector.tensor_reduce(se, ew, axis=mybir.AxisListType.X,
                                    op=ALU.add)
            rse = wp.tile([128, 1], F32, tag="rse")
            nc.vector.reciprocal(rse, se)
            nc.vector.tensor_scalar_mul(sent, pwa, rse)

        # ---- sent -> sentT + sentence input projections ----
        with tc.tile_pool(name="mid", bufs=1) as mp, \
             tc.tile_pool(name="pmid", bufs=1, space="PSUM") as pmp:
            ptm = pmp.tile([128, 512], BF, tag="ptm")
            for j in range(4):
                nc.tensor.transpose(ptm[:, j * 128:(j + 1) * 128],
                                    in_=sent[:, j * 128:(j + 1) * 128],
                                    identity=ident)
            sentT = mp.tile([128, 512], BF)
            nc.vector.tensor_copy(sentT[:, 0:256], ptm[:, 0:256])
            nc.scalar.copy(sentT[:, 256:512], ptm[:, 256:512])

            psg = pmp.tile([128, 1536], F32, tag="psg")
            for ns in range(3):
                sl = slice(ns * 512, (ns + 1) * 512)
                nc.tensor.matmul(psg[:, sl], lhsT=ones, rhs=sprow[:, sl],
                                 start=True, stop=False)
                for k in range(4):
                    nc.tensor.matmul(psg[:, sl],
                                     lhsT=sentT[:, k * 128:(k + 1) * 128],
                                     rhs=swihT[:, k * 1536 + ns * 512:
                                               k * 1536 + (ns + 1) * 512],
                                     start=False, stop=(k == 3))
            nc.scalar.copy(sgi[:, 0:512], psg[:, 0:512])
            nc.scalar.copy(sgi[:, 512:1024], psg[:, 512:1024])
            nc.vector.tensor_copy(sgi[:, 1024:1536], psg[:, 1024:1536])
            # transpose sgi into feature-major gate-chunk slots, grouped
            # per direction:
            # [d0: r0 r1 z0 z1 | d1: r0 r1 z0 z1 | d0: n0 n1 | d1: n0 n1]
            slot_src = [0, 128, 256, 384, 512, 640, 768, 896,
                        1024, 1152, 1280, 1408]
            ptg = pmp.tile([128, 1536], BF, tag="ptg")
            for j, src in enumerate(slot_src):
                nc.tensor.transpose(ptg[:, j * 128:(j + 1) * 128],
                                    in_=sgi[:, src:src + 128],
                                    identity=ident)
            psgT3c = psgT.rearrange("p (g r) -> p g r", g=12)
            ptg3c = ptg.rearrange("p (g r) -> p g r", g=12)
            nc.scalar.copy(psgT3c[:, :, 0:64], ptg3c[:, :, 0:64])
            nc.vector.tensor_copy(psgT3c[:, :, 64:128], ptg3c[:, :, 64:128])

        # ================= sentence stage =================
        # Feature-major sentence GRU: state h^T [128 feat(chunk d,m), 8 docs],
        # weight-stationary rec matmuls (N=8), gate math at FD<=32,
        # directions staggered so d1's matmuls stream during d0's gate math.
        with tc.tile_pool(name="sp", bufs=3) as sp:
          with tc.tile_pool(name="pgs", bufs=2, space="PSUM") as pgsp:
            psgT3 = psgT.rearrange("p (g r) -> p g r", g=12)

            for t in range(16):
                hprev = hsf_hist[:, t * 32:(t + 1) * 32]
                h_new = hsf_hist[:, (t + 1) * 32:(t + 2) * 32]
                prz_full = pgsp.tile([128, 512], F32, tag="prz")
                pnn_full = pgsp.tile([128, 512], F32, tag="pnn")
                prz = prz_full[:, 0:64]   # [d0: r0 r1 z0 z1 | d1: ...] x 8
                pnn = pnn_full[:, 0:32]   # [d0: n0 n1 | d1: n0 n1] x 8

                for d in range(2):
                    # --- PE: bias opener + recurrent matmuls, dir d ---
                    nc.tensor.matmul(pnn[:, d * 16:(d + 1) * 16], lhsT=sbrowF,
                                     rhs=bones[:, d * 16:(d + 1) * 16],
                                     start=True, stop=False)
                    for sl_ in range(4):  # (g in r,z) x (m in 0,1)
                        for k in range(2):
                            w_i = d * 8 + sl_ * 2 + k
                            w = swhhF[:, w_i * 128:(w_i + 1) * 128]
                            nc.tensor.matmul(
                                prz[:, d * 32 + sl_ * 8:d * 32 + (sl_ + 1) * 8],
                                lhsT=w,
                                rhs=hprev[:, (d * 2 + k) * 8:(d * 2 + k + 1) * 8],
                                start=(k == 0), stop=(k == 1))
                    for m in range(2):
                        for k in range(2):
                            w_i = 16 + d * 4 + m * 2 + k
                            w = swhhF[:, w_i * 128:(w_i + 1) * 128]
                            nc.tensor.matmul(
                                pnn[:, d * 16 + m * 8:d * 16 + (m + 1) * 8],
                                lhsT=w,
                                rhs=hprev[:, (d * 2 + k) * 8:(d * 2 + k + 1) * 8],
                                start=False, stop=(m == 1 and k == 1))

                    # --- gate math, dir d (tiny FD; runs while other dir's
                    #     matmuls stream on PE) ---
                    przd = prz[:, d * 32:(d + 1) * 32]
                    pnnd = pnn[:, d * 16:(d + 1) * 16]
                    hprevd = hprev[:, d * 16:(d + 1) * 16]
                    hnewd = h_new[:, d * 16:(d + 1) * 16]
                    rzp = sp.tile([128, 32], BF, tag=f"srzp{d}")
                    nc.vector.tensor_tensor(
                        rzp.rearrange("p (g r) -> p g r", g=4),
                        przd.rearrange("p (g r) -> p g r", g=4),
                        psgT3[:, d * 4:(d + 1) * 4, t * 8:(t + 1) * 8],
                        op=ALU.add)
                    rzd = sp.tile([128, 32], BF, tag=f"srz{d}")
                    nc.scalar.activation(rzd, rzp, AF.Sigmoid)
                    t1 = sp.tile([128, 16], BF, tag=f"st1{d}")
                    nc.vector.tensor_tensor(t1, rzd[:, 0:16], pnnd, op=ALU.mult)
                    npre = sp.tile([128, 16], BF, tag=f"snp{d}")
                    nc.vector.tensor_tensor(
                        npre.rearrange("p (g r) -> p g r", g=2),
                        t1.rearrange("p (g r) -> p g r", g=2),
                        psgT3[:, 8 + d * 2:8 + (d + 1) * 2, t * 8:(t + 1) * 8],
                        op=ALU.add)
                    omz = sp.tile([128, 16], BF, tag=f"som{d}")
                    nc.gpsimd.tensor_scalar(out=omz, in0=rzd[:, 16:32],
                                            scalar1=-1.0, scalar2=1.0,
                                            op0=ALU.mult, op1=ALU.add)
                    zh = sp.tile([128, 16], BF, tag=f"szh{d}")
                    nc.gpsimd.tensor_tensor(zh, rzd[:, 16:32], hprevd,
                                            op=ALU.mult)
                    nnT = sp.tile([128, 16], BF, tag=f"snn{d}")
                    nc.scalar.activation(nnT, npre, AF.Tanh)
                    nom = sp.tile([128, 16], BF, tag=f"snm{d}")
                    nc.vector.tensor_tensor(nom, nnT, omz, op=ALU.mult)
                    nc.vector.tensor_add(hnewd, nom, zh)

                    # batch layout for the attention post-pass
                    hsbT4 = hsbT.rearrange("p (d c r) -> p d c r", d=2, c=2)
                    nc.scalar.copy(
                        hsbT4[:, d, :, 8 * t:8 * (t + 1)],
                        hnewd.rearrange("p (c r) -> p c r", c=2))

            # rebuild batch-major hsb from hsbT (4 transposes)
            pth = pgsp.tile([128, 512], BF, tag="pth")
            for j in range(4):
                nc.tensor.transpose(pth[:, j * 128:(j + 1) * 128],
                                    in_=hsbT[:, j * 128:(j + 1) * 128],
                                    identity=ident)
            nc.vector.tensor_copy(hsb[:, 0:256], pth[:, 0:256])
            nc.scalar.copy(hsb[:, 256:512], pth[:, 256:512])

          # ---- sentence attention post-pass (batch rows = (s,d)) ----
          with tc.tile_pool(name="pps", bufs=1, space="PSUM") as ppsp:
            pu2 = ppsp.tile([128, 512], F32, tag="pu2")
            nc.tensor.matmul(pu2, lhsT=ones, rhs=sbarow, start=True, stop=False)
            for j in range(4):
                nc.tensor.matmul(pu2, lhsT=hsbT[:, j * 128:(j + 1) * 128],
                                 rhs=sawT[:, j * 512:(j + 1) * 512],
                                 start=False, stop=(j == 3))
            u2 = sp.tile([128, 512], BF, tag="u2")
            nc.scalar.activation(u2, pu2, AF.Tanh)
            s_sc = sp.tile([128, 1], F32, tag="s_sc")
            scr2 = sp.tile([128, 512], BF, tag="scr2")
            nc.vector.scalar_tensor_tensor(
                out=scr2, in0=u2, scalar=1.0, in1=svb,
                op0=ALU.mult, op1=ALU.mult, accum_out=s_sc)
            # exp via sigmoid set: exp(s) = 1/sigmoid(-s) - 1
            ew2s = sp.tile([128, 1], F32, tag="ew2s")
            nc.scalar.activation(ew2s, s_sc, AF.Sigmoid, scale=-1.0)
            ew2r = sp.tile([128, 1], F32, tag="ew2r")
            nc.vector.reciprocal(ew2r, ew2s)
            ew2 = sp.tile([128, 1], F32, tag="ew2")
            nc.vector.tensor_scalar_add(ew2, ew2r, -1.0)
            # per-doc sums via indicator matmul, then broadcast back
            pds = ppsp.tile([8, 1], F32, tag="pds")
            nc.tensor.matmul(pds, lhsT=ind8f, rhs=ew2, start=True, stop=True)
            dsum = sp.tile([8, 1], F32, tag="dsum")
            nc.vector.tensor_copy(dsum, pds)
            rds = sp.tile([8, 1], F32, tag="rds")
            nc.vector.reciprocal(rds, dsum)
            prb = ppsp.tile([128, 1], F32, tag="prb")
            nc.tensor.matmul(prb, lhsT=ind8T, rhs=rds, start=True, stop=True)
            aw = sp.tile([128, 1], F32, tag="aw")
            nc.vector.tensor_tensor(aw, ew2, prb, op=ALU.mult)
            awd = sp.tile([128, 8], BF, tag="awd")
            nc.vector.tensor_scalar_mul(awd, ind8, aw)
            pdoc = ppsp.tile([8, 512], F32, tag="pdoc")
            nc.tensor.matmul(pdoc, lhsT=awd, rhs=hsb, start=True, stop=True)
            doc = sp.tile([8, 512], BF, tag="doc")
            nc.scalar.copy(doc, pdoc)

            # ---- classifier + log_softmax ----
            ptd = ppsp.tile([128, 32], BF, tag="ptd")
            for j in range(4):
                nc.tensor.transpose(ptd[:, j * 8:(j + 1) * 8],
                                    in_=doc[:, j * 128:(j + 1) * 128],
                                    identity=ident[0:8, 0:8])
            docT = sp.tile([128, 32], BF, tag="docT")
            nc.vector.tensor_copy(docT, ptd)
            pl = ppsp.tile([8, NCLS], F32, tag="pl")
            nc.tensor.matmul(pl, lhsT=ones[:, 0:8], rhs=fcb,
                             start=True, stop=False)
            for j in range(4):
                nc.tensor.matmul(pl, lhsT=docT[:, j * 8:(j + 1) * 8],
                                 rhs=fcwT[:, j * NCLS:(j + 1) * NCLS],
                                 start=False, stop=(j == 3))
            nmx2 = sp.tile([8, 1], F32, tag="nmx2")
            nc.vector.tensor_reduce(nmx2, pl, axis=mybir.AxisListType.X,
                                    op=ALU.max, negate=True)
            mx2p = sp.tile([8, 1], F32, tag="mx2p")
            nc.vector.tensor_scalar_mul(mx2p, nmx2, -1.0)
            # exp(pl - max) = 1/sigmoid(-(pl - max)) - 1: stays in the
            # sigmoid table set, avoiding the exp_and_others table load
            e2s = sp.tile([8, NCLS], F32, tag="e2s")
            nc.scalar.activation(e2s, pl, AF.Sigmoid, scale=-1.0, bias=mx2p)
            e2r = sp.tile([8, NCLS], F32, tag="e2r")
            nc.vector.reciprocal(e2r, e2s)
            e2 = sp.tile([8, NCLS], F32, tag="e2")
            nc.vector.tensor_scalar_add(e2, e2r, -1.0)
            se2 = sp.tile([8, 1], F32, tag="se2")
            nc.vector.tensor_reduce(se2, e2, axis=mybir.AxisListType.X,
                                    op=ALU.add)
            lse = sp.tile([8, 1], F32, tag="lse")
            nc.scalar.activation(lse, se2, AF.Ln)
            out_sb = sp.tile([8, NCLS], F32, tag="out_sb")
            nc.vector.tensor_scalar(out=out_sb, in0=pl, scalar1=nmx2,
                                    scalar2=lse, op0=ALU.add, op1=ALU.subtract)
            nc.sync.dma_start(out=dram("out"), in_=out_sb)


# ---------------------------------------------------------------------------
# host side
# ---------------------------------------------------------------------------

def _prep_inputs(inputs):
    """Build the per-core in_maps (host preprocessing + sharding)."""
    f32 = np.float32
    emb = np.asarray(inputs["emb"], f32)
    w_Wih = np.asarray(inputs["w_Wih"], f32)
    w_Whh = np.asarray(inputs["w_Whh"], f32)
    w_bih = np.asarray(inputs["w_bih"], f32)
    w_bhh = np.asarray(inputs["w_bhh"], f32)
    wa_W = np.asarray(inputs["wa_W"], f32)
    wa_b = np.asarray(inputs["wa_b"], f32)
    wa_v = np.asarray(inputs["wa_v"], f32)
    s_Wih = np.asarray(inputs["s_Wih"], f32)
    s_Whh = np.asarray(inputs["s_Whh"], f32)
    s_bih = np.asarray(inputs["s_bih"], f32)
    s_bhh = np.asarray(inputs["s_bhh"], f32)
    sa_W = np.asarray(inputs["sa_W"], f32)
    sa_b = np.asarray(inputs["sa_b"], f32)
    sa_v = np.asarray(inputs["sa_v"], f32)
    fc_W = np.asarray(inputs["fc_W"], f32)
    fc_b = np.asarray(inputs["fc_b"], f32)
    tokens = np.asarray(inputs["tokens"])

    def b(x):
        return np.ascontiguousarray(x.astype(bf16))

    # folded gather table G [V, 1536] = [rz0 | rz1 | n0 | n1]
    g0 = emb @ w_Wih[0].T + w_bih[0]
    g0[:, :512] += w_bhh[0][:512]
    g1 = emb @ w_Wih[1].T + w_bih[1]
    g1[:, :512] += w_bhh[1][:512]
    G = np.concatenate([g0[:, :512], g1[:, :512], g0[:, 512:], g1[:, 512:]], 1)

    whhT = np.stack([w_Whh[0].T[:128], w_Whh[0].T[128:],
                     w_Whh[1].T[:128], w_Whh[1].T[128:]])  # [4,128,768]
    brow = np.concatenate([w_bhh[0][512:], w_bhh[1][512:]])[None, :]
    vbc = np.broadcast_to(wa_v, (128, 512))

    # sentence input-proj table [512, 1536] with same col layout; bias row
    sg0 = s_Wih[0].T  # [512, 768]
    sg1 = s_Wih[1].T
    swihT = np.concatenate([sg0[:, :512], sg1[:, :512],
                            sg0[:, 512:], sg1[:, 512:]], 1)
    sprow = np.concatenate([
        s_bih[0][:512] + s_bhh[0][:512],
        s_bih[1][:512] + s_bhh[1][:512],
        s_bih[0][512:], s_bih[1][512:]])[None, :]
    # feature-major weight chunks for the sentence GRU, slot order grouped
    # by direction: rz: for d: for g in (r,z): for m: for k;
    # n: for d: for m: for k
    goff = {"r": 0, "z": 256, "n": 512}
    fch = []
    for d in range(2):
        for g in ("r", "z"):
            for m in range(2):
                for k in range(2):
                    fch.append(s_Whh[d][goff[g] + m * 128:goff[g] + (m + 1) * 128,
                                        k * 128:(k + 1) * 128].T)
    for d in range(2):
        for m in range(2):
            for k in range(2):
                fch.append(s_Whh[d][goff["n"] + m * 128:goff["n"] + (m + 1) * 128,
                                    k * 128:(k + 1) * 128].T)
    swhhF = np.stack(fch)  # [24, 128, 128]
    sbrow = np.concatenate([s_bhh[0][512:], s_bhh[1][512:]])[None, :]
    svbc = np.broadcast_to(sa_v, (128, 512))

    ind = np.zeros((128, 8), f32)
    for row in range(128):
        ind[row, row % 8] = 1.0

    shared = {
        "G": b(G), "whhT": b(whhT), "brow": b(brow),
        "waT": b(wa_W.T), "barow": b(wa_b[None, :]), "vb": b(vbc),
        "swihT": b(swihT), "sprow": b(sprow),
        "swhhF": b(swhhF),
        "sbrowF": b(sbrow.reshape(4, 128)),
        "bones": b(np.repeat(np.eye(4, dtype=f32), 8, axis=1)),
        "sbrow": b(sbrow), "sawT": b(sa_W.T), "sbarow": b(sa_b[None, :]),
        "svb": b(svbc), "fcwT": b(fc_W.T), "fcb": b(fc_b[None, :]),
        "ind8": b(ind), "ind8f": np.ascontiguousarray(ind),
        "ind8T": np.ascontiguousarray(ind.T),
    }
    in_maps = []
    for c in range(NCORES):
        # word-row p = s*8 + doc  (so sentence step s owns partition rows
        # [s*8:(s+1)*8] of the batch-major sentence matrix)
        tk = np.ascontiguousarray(
            np.transpose(tokens[c * BC:(c + 1) * BC], (1, 0, 2))
            .reshape(NW, W).astype(np.int32))
        in_maps.append({**shared, "toks": tk})
    return in_maps


_NC_CACHE = {}


def _get_nc():
    if "nc" not in _NC_CACHE:
        _NC_CACHE["nc"] = _build_program()
    return _NC_CACHE["nc"]


def kernel(**inputs) -> np.ndarray:
    nc = _get_nc()
    in_maps = _prep_inputs(inputs)
    res = bass_utils.run_bass_kernel_spmd(nc, in_maps, core_ids=list(range(NCORES)))
    outs = []
    for c in range(NCORES):
        o = np.asarray(res.results[c]["out"], np.float32)
        outs.append(o)
    return np.concatenate(outs, 0)
